# revision 28
# baseline (speedup 1.0000x reference)
"""Trainium2 Bass kernel for ActionExpertAttention (dense transformer block).

Strategy: data-parallel over batch (16 batches -> 2 per core on 8 cores).
All matmuls run in bf16 with fp32 PSUM accumulation. The whole pipeline is
computed in "transposed" space so nothing needs an on-chip transpose except
V_new (16 small PE transposes):

  qkv^T[n, m]   = wqkv^T_chunk^T . hs^T          (n-chunks of 128)
  scores^T[k,q] = Krot^T_chunk^T . Qrot^T        (kv-chunks of 128)
  out^T[d, q]   = V_chunk^T      . exp(scores^T) (accumulated over kv)
  final[q, n]   = attn^T_chunk^T . wo^T          (accumulated over heads)

Softmax denominators come from ones-vector matmuls over exp(scores^T) that
run 4-at-a-time in the PE array via column tiling (tile_position), are
combined with partition-shifted DVE adds, deferred-inverted with sliced
Ln + Exp(-x) passes on ACT (2 activation-table loads total instead of 32),
broadcast across partitions with a tiny K=1 outer-product matmul, and applied
in-place to the unnormalized attn^T. RoPE rotate-half uses sign-baked sin
tables and a host-prepared half-swapped K copy streamed from HBM. All large
DMAs are host-reordered to be fully contiguous.
"""

import sys

sys.path.insert(0, "/opt/trn_rl_repo")

import numpy as np
import ml_dtypes

import concourse.bass as bass
import concourse.tile as tile
from concourse import mybir, bacc
from concourse.bass_utils import run_bass_kernel_spmd
from concourse.masks import make_identity

BF = ml_dtypes.bfloat16

B, Q, VLM = 16, 128, 2048
H, HKV, D = 16, 8, 128
HID = H * D            # 2048
G = H // HKV           # 2
KV = VLM + Q           # 2176
THETA = 10000.0
N_CORES = 8
B_LOC = B // N_CORES   # 2
KDIM = HID
NQKV = (H + 2 * HKV) * D  # 4096
KO = KDIM // 128       # 16
NCH = NQKV // 128      # 32
KVCH = KV // 128       # 17
M = B_LOC * Q          # 256
NGRP = B_LOC * HKV     # 16
WOT = 256              # wo n-tile width
NWT = HID // WOT       # 8

f32 = mybir.dt.float32
bf16 = mybir.dt.bfloat16


def _build_nc(repeat=1):
    nc = bacc.Bacc(trn_type="TRN2", num_swdge_queues=4)

    # all big inputs host-reordered so every DMA is fully contiguous
    hsT_d = nc.dram_tensor("hsT", [128, KO, M], bf16, kind="ExternalInput")
    wq_d = nc.dram_tensor("wqr", [NCH, 128, KO, 128], bf16, kind="ExternalInput")
    wo_d = nc.dram_tensor("wor", [NWT, 128, H, WOT], bf16, kind="ExternalInput")
    vkT_d = nc.dram_tensor("vkT", [B_LOC, HKV, D, VLM], bf16, kind="ExternalInput")
    vkTs_d = nc.dram_tensor("vkTs", [B_LOC, HKV, D, VLM], bf16, kind="ExternalInput")
    vv_d = nc.dram_tensor("vvr", [B_LOC, HKV, 128, KO, D], bf16, kind="ExternalInput")
    maskT_d = nc.dram_tensor("maskT", [Q, B_LOC, Q], f32, kind="ExternalInput")
    cos_d = nc.dram_tensor("cosT", [D, KV], bf16, kind="ExternalInput")
    sins_d = nc.dram_tensor("sinTs", [D, KV], bf16, kind="ExternalInput")
    cosq_d = nc.dram_tensor("cosqT", [D, B_LOC, Q], bf16, kind="ExternalInput")
    sinq_d = nc.dram_tensor("sinqTs", [D, B_LOC, Q], bf16, kind="ExternalInput")
    out_d = nc.dram_tensor("out", [B_LOC, Q, HID], f32, kind="ExternalOutput")

    from contextlib import ExitStack
    with tile.TileContext(nc) as tc, ExitStack() as ctx:
        const = ctx.enter_context(tc.tile_pool(name="const", bufs=1))
        wqp = ctx.enter_context(tc.tile_pool(name="wq", bufs=5))
        wop = ctx.enter_context(tc.tile_pool(name="wo", bufs=3))
        ktp = ctx.enter_context(tc.tile_pool(name="kt", bufs=3))
        ktsp = ctx.enter_context(tc.tile_pool(name="kts", bufs=3))
        krotp = ctx.enter_context(tc.tile_pool(name="krot", bufs=3))
        ktmpp = ctx.enter_context(tc.tile_pool(name="ktmp", bufs=2))
        vvp = ctx.enter_context(tc.tile_pool(name="vv", bufs=2))
        expp = ctx.enter_context(tc.tile_pool(name="expp", bufs=2))
        tmp = ctx.enter_context(tc.tile_pool(name="tmp", bufs=6))
        outp = ctx.enter_context(tc.tile_pool(name="outp", bufs=2))
        ps = ctx.enter_context(tc.tile_pool(name="ps", bufs=1, space="PSUM"))

        # one-time constants
        ones_sb = const.tile([128, 1], bf16, tag="ones")
        nc.vector.memset(ones_sb, 1.0)
        onesrow_sb = const.tile([1, 128], bf16, tag="onesrow")
        nc.vector.memset(onesrow_sb, 1.0)
        id_sb = const.tile([128, 128], bf16, tag="ident")
        make_identity(nc, id_sb)

        def emit_body():
            # ------- startup loads (first weight tiles prefetched) -------
            wq_tiles = {}
            for nch in range(2):
                wq = wqp.tile([128, KO, 128], bf16, tag="wq")
                nc.sync.dma_start(out=wq, in_=wq_d[nch])
                wq_tiles[nch] = wq
            hs_sb = const.tile([128, KO, M], bf16, tag="hs")
            nc.sync.dma_start(out=hs_sb, in_=hsT_d[:])

            cos_sb = const.tile([128, KV], bf16, tag="cos")
            nc.sync.dma_start(out=cos_sb, in_=cos_d[:])
            sins_sb = const.tile([128, KV], bf16, tag="sins")
            nc.sync.dma_start(out=sins_sb, in_=sins_d[:])
            cosq_sb = const.tile([128, B_LOC, Q], bf16, tag="cosq")
            nc.sync.dma_start(out=cosq_sb, in_=cosq_d[:])
            sinq_sb = const.tile([128, B_LOC, Q], bf16, tag="sinq")
            nc.sync.dma_start(out=sinq_sb, in_=sinq_d[:])

            maskT_sb = const.tile([128, B_LOC, Q], f32, tag="maskT")
            nc.sync.dma_start(out=maskT_sb, in_=maskT_d[:])
            qT_sb = const.tile([128, B_LOC, H, Q], bf16, tag="qT")
            knT_sb = const.tile([128, B_LOC, HKV, Q], bf16, tag="knT")
            vn_sb = const.tile([128, B_LOC, HKV, D], bf16, tag="vn")
            attnT_sb = const.tile([128, B_LOC, H, Q], bf16, tag="attnT")
            sums_sb = const.tile([1, NGRP, M], f32, tag="sums")
            rec_all = const.tile([1, NGRP, M], bf16, tag="recall")

            def rope_from_psum(seg, cos_ap, sins_ap, out_ap):
                w = seg.shape[-1]
                tcos = tmp.tile([128, w], bf16, tag="tcos")
                nc.vector.tensor_tensor(out=tcos, in0=seg, in1=cos_ap, op=mybir.AluOpType.mult)
                tsin = tmp.tile([128, w], bf16, tag="tsin")
                nc.vector.tensor_tensor(
                    out=tsin[0:64, :], in0=seg[64:128, :], in1=sins_ap[0:64, :],
                    op=mybir.AluOpType.mult,
                )
                nc.vector.tensor_tensor(
                    out=tsin[64:128, :], in0=seg[0:64, :], in1=sins_ap[64:128, :],
                    op=mybir.AluOpType.mult,
                )
                nc.vector.tensor_tensor(out=out_ap, in0=tcos, in1=tsin, op=mybir.AluOpType.add)

            # ---------- phase 1: qkv^T projection ----------
            for nch in range(NCH):
                hkv, slot = nch // 4, nch % 4
                if nch in wq_tiles:
                    wq = wq_tiles.pop(nch)
                else:
                    wq = wqp.tile([128, KO, 128], bf16, tag="wq")
                    nc.sync.dma_start(out=wq, in_=wq_d[nch])
                pq = ps.tile([128, M], f32, tag="qkv", bufs=2)
                for ko in range(KO):
                    nc.tensor.matmul(
                        pq, wq[:, ko, :], hs_sb[:, ko, :],
                        start=(ko == 0), stop=(ko == KO - 1),
                    )
                if slot <= 1:
                    h = hkv * G + slot
                    for b in range(B_LOC):
                        rope_from_psum(
                            pq[:, b * Q:(b + 1) * Q],
                            cosq_sb[:, b, :], sinq_sb[:, b, :],
                            qT_sb[:, b, h, :],
                        )
                elif slot == 2:
                    for b in range(B_LOC):
                        rope_from_psum(
                            pq[:, b * Q:(b + 1) * Q],
                            cos_sb[:, VLM:VLM + Q], sins_sb[:, VLM:VLM + Q],
                            knT_sb[:, b, hkv, :],
                        )
                else:
                    vt = tmp.tile([128, M], bf16, tag="vt", bufs=3)
                    nc.vector.tensor_copy(out=vt, in_=pq)
                    for b in range(B_LOC):
                        pvt = ps.tile([128, 128], bf16, tag="qkv", bufs=2)
                        nc.tensor.transpose(pvt, vt[:, b * Q:(b + 1) * Q], id_sb)
                        nc.vector.tensor_copy(out=vn_sb[:, b, hkv, :], in_=pvt)

            # ---------- phase 2: attention per (b, hkv) ----------
            for b in range(B_LOC):
                for hkv in range(HKV):
                    grp = b * HKV + hkv
                    kt = ktp.tile([128, VLM], bf16, tag="kt")
                    nc.sync.dma_start(out=kt, in_=vkT_d[b, hkv])
                    kts = ktsp.tile([128, VLM], bf16, tag="kts")
                    nc.sync.dma_start(out=kts, in_=vkTs_d[b, hkv])
                    vvt = vvp.tile([128, KO, D], bf16, tag="vv")
                    nc.sync.dma_start(out=vvt, in_=vv_d[b, hkv])

                    krot = krotp.tile([128, VLM], bf16, tag="krot")
                    nc.vector.tensor_tensor(out=krot, in0=kt, in1=cos_sb[:, 0:VLM], op=mybir.AluOpType.mult)
                    ktmp = ktmpp.tile([128, VLM], bf16, tag="ktmp")
                    nc.vector.tensor_tensor(out=ktmp, in0=kts, in1=sins_sb[:, 0:VLM], op=mybir.AluOpType.mult)
                    nc.vector.tensor_tensor(out=krot, in0=krot, in1=ktmp, op=mybir.AluOpType.add)

                    qT_ap = qT_sb[:, b, hkv * G:(hkv + 1) * G, :]
                    expT = expp.tile([128, KVCH, M], bf16, tag="expT")

                    for cc in range((KVCH + 1) // 2):
                        c0 = cc * 2
                        npair = 2 if c0 + 1 < KVCH else 1
                        pqk = ps.tile([128, 512], f32, tag="qk", bufs=3)
                        for half in range(npair):
                            c = c0 + half
                            lhsT = krot[:, c * 128:(c + 1) * 128] if c < VLM // 128 \
                                else knT_sb[:, b, hkv, :]
                            nc.tensor.matmul(
                                pqk[:, half * M:(half + 1) * M], lhsT, qT_ap,
                                start=True, stop=True,
                            )
                            if c == KVCH - 1:
                                mask_b = maskT_sb[:, b, :]
                                mask_bc = bass.AP(
                                    tensor=mask_b.tensor, offset=mask_b.offset,
                                    ap=[mask_b.ap[0], [0, G], mask_b.ap[1]],
                                )
                                seg = pqk[:, half * M:(half + 1) * M]
                                nc.vector.tensor_tensor(out=seg, in0=seg, in1=mask_bc, op=mybir.AluOpType.add)
                        nc.scalar.activation(
                            out=expT[:, c0:c0 + npair, :], in_=pqk[:, 0:npair * M],
                            func=mybir.ActivationFunctionType.Exp,
                        )

                    po = ps.tile([128, M], f32, tag="pv", bufs=2)
                    psum_s = ps.tile([128, M], f32, tag="sum", bufs=1)
                    for c in range(KVCH):
                        lhsT = vvt[:, c, :] if c < VLM // 128 else vn_sb[:, b, hkv, :]
                        nc.tensor.matmul(po, lhsT, expT[:, c, :], start=(c == 0), stop=(c == KVCH - 1))
                    # exp-sums: 4 col-tiled M=1 accumulations run concurrently in the
                    # PE array; chunk-major order so adjacent MMs hit different col groups
                    for c in range(KVCH):
                        j = c % 4
                        nc.tensor.matmul(
                            psum_s[32 * j:32 * j + 1, :], ones_sb, expT[:, c, :],
                            start=(c < 4), stop=(c >= KVCH - 4),
                            tile_position=(0, 32 * j),
                        )
                    nc.vector.tensor_copy(out=attnT_sb[:, b, hkv * G:(hkv + 1) * G, :], in_=po)
                    scomb = tmp.tile([1, M], f32, tag="scomb", bufs=2)
                    nc.vector.tensor_copy(out=scomb, in_=psum_s[0:1, :])
                    for j in (1, 2):
                        nc.vector.tensor_tensor(out=scomb, in0=psum_s[32 * j:32 * j + 1, :], in1=scomb, op=mybir.AluOpType.add)
                    nc.vector.tensor_tensor(out=sums_sb[:, grp, :], in0=psum_s[96:97, :], in1=scomb, op=mybir.AluOpType.add)

            # ------- softmax normalization (deferred; 2 ACT table loads) ----
            lns = tmp.tile([1, NGRP, M], f32, tag="lns", bufs=1)
            NSL = 4
            for sl in range(NSL):
                g0, g1 = sl * (NGRP // NSL), (sl + 1) * (NGRP // NSL)
                nc.scalar.activation(out=lns[:, g0:g1, :], in_=sums_sb[:, g0:g1, :],
                                     func=mybir.ActivationFunctionType.Ln)
                nc.scalar.activation(out=rec_all[:, g0:g1, :], in_=lns[:, g0:g1, :],
                                     func=mybir.ActivationFunctionType.Exp, scale=-1.0)
            for b in range(B_LOC):
                for hkv in range(HKV):
                    grp = b * HKV + hkv
                    prec = ps.tile([128, M], f32, tag="pv", bufs=2)
                    nc.tensor.matmul(prec, onesrow_sb, rec_all[:, grp, :], start=True, stop=True)
                    rec128 = tmp.tile([128, M], bf16, tag="rec128", bufs=2)
                    nc.vector.tensor_copy(out=rec128, in_=prec)
                    at = attnT_sb[:, b, hkv * G:(hkv + 1) * G, :]
                    nc.vector.tensor_tensor(out=at, in0=at, in1=rec128, op=mybir.AluOpType.mult)

            # ---------- phase 3: output projection ----------
            for nt in range(NWT):
                wo_t = wop.tile([128, H, WOT], bf16, tag="wo")
                nc.sync.dma_start(out=wo_t, in_=wo_d[nt])
                for b in range(B_LOC):
                    pw = ps.tile([128, WOT], f32, tag="qk", bufs=3)
                    for h in range(H):
                        nc.tensor.matmul(
                            pw, attnT_sb[:, b, h, :], wo_t[:, h, :],
                            start=(h == 0), stop=(h == H - 1),
                        )
                    ot = outp.tile([128, WOT], f32, tag="ot")
                    nc.vector.tensor_copy(out=ot, in_=pw)
                    nc.sync.dma_start(out=out_d[b, :, nt * WOT:(nt + 1) * WOT], in_=ot)

        for _rep in range(repeat):
            emit_body()

    nc.finalize()
    return nc


_NC_CACHE = None


def _get_nc():
    global _NC_CACHE
    if _NC_CACHE is None:
        _NC_CACHE = _build_nc()
    return _NC_CACHE


def _host_prep(hidden_states, vlm_key, vlm_value, position_ids, attention_mask,
               wqkv_w, wo_w):
    hs = np.asarray(hidden_states, dtype=np.float32)
    vk = np.asarray(vlm_key, dtype=np.float32)
    vv = np.asarray(vlm_value, dtype=np.float32)
    pos = np.asarray(position_ids).astype(np.int64)
    am = np.asarray(attention_mask, dtype=np.float32)
    wqkv = np.asarray(wqkv_w, dtype=np.float32)
    wo = np.asarray(wo_w, dtype=np.float32)

    # wqkv^T reordered: (nch, ki, ko, nj) fully contiguous per chunk
    wqkvT = wqkv.T.astype(BF)                                  # (2048, 4096)
    wq_r = np.ascontiguousarray(
        wqkvT.reshape(KO, 128, NCH, 128).transpose(2, 1, 0, 3))  # (32,128,16,128)
    # wo^T reordered: (nt, d, h, n)
    woT = wo.T.astype(BF)                                      # (hd, n)
    wo_r = np.ascontiguousarray(
        woT.reshape(H, 128, NWT, WOT).transpose(2, 1, 0, 3))     # (8,128,16,256)

    inv = 1.0 / (THETA ** (np.arange(0, D, 2, dtype=np.float32) / D))
    t = np.arange(KV, dtype=np.float32)
    fr = np.outer(t, inv)
    emb = np.concatenate([fr, fr], axis=-1)
    cosT = np.ascontiguousarray(np.cos(emb).T)       # (D, KV) fp32
    sinT = np.ascontiguousarray(np.sin(emb).T)
    sinTs = sinT.copy()
    sinTs[: D // 2] *= -1.0
    scale = 1.0 / np.sqrt(np.float32(D))

    in_maps = []
    for core in range(N_CORES):
        bs = slice(core * B_LOC, (core + 1) * B_LOC)
        hsT_i = np.ascontiguousarray(
            hs[bs].transpose(2, 0, 1).reshape(KO, 128, M).transpose(1, 0, 2)
        ).astype(BF)                                  # (128, 16, 256)
        vkT_i = np.ascontiguousarray(vk[bs].transpose(0, 1, 3, 2)).astype(BF)
        vkTs_i = np.ascontiguousarray(
            np.concatenate([vkT_i[:, :, D // 2:, :], vkT_i[:, :, : D // 2, :]], axis=2))
        vv_i = np.ascontiguousarray(
            vv[bs].reshape(B_LOC, HKV, KO, 128, D).transpose(0, 1, 3, 2, 4)
        ).astype(BF)                                  # (2,8,128,16,128)
        maskT_i = np.ascontiguousarray(
            np.maximum(am[bs, 0, :, VLM:], -30.0).transpose(2, 0, 1)
        ).astype(np.float32)
        posq = pos[bs] + KV - Q
        cosq_i = np.ascontiguousarray((cosT[:, posq] * scale)).astype(BF)   # (128,2,128)
        sinq_i = np.ascontiguousarray((sinTs[:, posq] * scale)).astype(BF)
        in_maps.append({
            "hsT": hsT_i,
            "wqr": wq_r,
            "wor": wo_r,
            "vkT": vkT_i,
            "vkTs": vkTs_i,
            "vvr": vv_i,
            "maskT": maskT_i,
            "cosT": cosT.astype(BF),
            "sinTs": sinTs.astype(BF),
            "cosqT": cosq_i,
            "sinqTs": sinq_i,
        })
    return in_maps


def kernel(hidden_states, vlm_key, vlm_value, position_ids, attention_mask,
           wqkv_w, wo_w, _trace=False):
    nc = _get_nc()
    in_maps = _host_prep(hidden_states, vlm_key, vlm_value, position_ids,
                         attention_mask, wqkv_w, wo_w)
    res = run_bass_kernel_spmd(nc, in_maps, core_ids=list(range(N_CORES)), trace=_trace)
    out = np.concatenate([res.results[i]["out"] for i in range(N_CORES)], axis=0)
    if _trace:
        kernel._last_results = res
    return out.astype(np.float32)


if __name__ == "__main__":
    rng = np.random.default_rng(0)
    ins = {
        "hidden_states": rng.standard_normal((B, Q, HID), dtype=np.float32),
        "vlm_key": rng.standard_normal((B, HKV, VLM, D), dtype=np.float32),
        "vlm_value": rng.standard_normal((B, HKV, VLM, D), dtype=np.float32),
        "position_ids": np.tile(np.arange(Q, dtype=np.int32), (B, 1)),
        "attention_mask": np.zeros((B, 1, Q, KV), dtype=np.float32),
        "wqkv_w": rng.standard_normal((NQKV, HID), dtype=np.float32) * 0.02,
        "wo_w": rng.standard_normal((HID, HID), dtype=np.float32) * 0.02,
    }
    out = kernel(**ins)
    print("out", out.shape, out.dtype, float(np.abs(out).max()))


# revision 29
# speedup vs baseline: 1.0008x; 1.0008x over previous
"""Trainium2 Bass kernel for ActionExpertAttention (dense transformer block).

Strategy: data-parallel over batch (16 batches -> 2 per core on 8 cores).
All matmuls run in bf16 with fp32 PSUM accumulation. The whole pipeline is
computed in "transposed" space so nothing needs an on-chip transpose except
V_new (16 small PE transposes):

  qkv^T[n, m]   = wqkv^T_chunk^T . hs^T          (n-chunks of 128)
  scores^T[k,q] = Krot^T_chunk^T . Qrot^T        (kv-chunks of 128)
  out^T[d, q]   = V_chunk^T      . exp(scores^T) (accumulated over kv)
  final[q, n]   = attn^T_chunk^T . wo^T          (accumulated over heads)

Softmax denominators come from ones-vector matmuls over exp(scores^T) that
run 4-at-a-time in the PE array via column tiling (tile_position), are
combined with partition-shifted DVE adds, deferred-inverted with sliced
Ln + Exp(-x) passes on ACT (2 activation-table loads total instead of 32),
broadcast across partitions with a tiny K=1 outer-product matmul, and applied
in-place to the unnormalized attn^T. RoPE rotate-half uses sign-baked sin
tables and a host-prepared half-swapped K copy streamed from HBM. All large
DMAs are host-reordered to be fully contiguous.
"""

import sys

sys.path.insert(0, "/opt/trn_rl_repo")

import numpy as np
import ml_dtypes

import concourse.bass as bass
import concourse.tile as tile
from concourse import mybir, bacc
from concourse.bass_utils import run_bass_kernel_spmd
from concourse.masks import make_identity

BF = ml_dtypes.bfloat16

B, Q, VLM = 16, 128, 2048
H, HKV, D = 16, 8, 128
HID = H * D            # 2048
G = H // HKV           # 2
KV = VLM + Q           # 2176
THETA = 10000.0
N_CORES = 8
B_LOC = B // N_CORES   # 2
KDIM = HID
NQKV = (H + 2 * HKV) * D  # 4096
KO = KDIM // 128       # 16
NCH = NQKV // 128      # 32
KVCH = KV // 128       # 17
M = B_LOC * Q          # 256
NGRP = B_LOC * HKV     # 16
WOT = 256              # wo n-tile width
NWT = HID // WOT       # 8

f32 = mybir.dt.float32
bf16 = mybir.dt.bfloat16


def _build_nc(repeat=1):
    nc = bacc.Bacc(trn_type="TRN2", num_swdge_queues=4)

    # all big inputs host-reordered so every DMA is fully contiguous
    hsT_d = nc.dram_tensor("hsT", [128, KO, M], bf16, kind="ExternalInput")
    wq_d = nc.dram_tensor("wqr", [NCH, 128, KO, 128], bf16, kind="ExternalInput")
    wo_d = nc.dram_tensor("wor", [NWT, 128, H, WOT], bf16, kind="ExternalInput")
    vkT_d = nc.dram_tensor("vkT", [B_LOC, HKV, D, VLM], bf16, kind="ExternalInput")
    vkTs_d = nc.dram_tensor("vkTs", [B_LOC, HKV, D, VLM], bf16, kind="ExternalInput")
    vv_d = nc.dram_tensor("vvr", [B_LOC, HKV, 128, KO, D], bf16, kind="ExternalInput")
    maskT_d = nc.dram_tensor("maskT", [Q, B_LOC, Q], f32, kind="ExternalInput")
    cos_d = nc.dram_tensor("cosT", [D, KV], bf16, kind="ExternalInput")
    sins_d = nc.dram_tensor("sinTs", [D, KV], bf16, kind="ExternalInput")
    cosq_d = nc.dram_tensor("cosqT", [D, B_LOC, Q], bf16, kind="ExternalInput")
    sinq_d = nc.dram_tensor("sinqTs", [D, B_LOC, Q], bf16, kind="ExternalInput")
    out_d = nc.dram_tensor("out", [B_LOC, Q, HID], f32, kind="ExternalOutput")

    from contextlib import ExitStack
    with tile.TileContext(nc) as tc, ExitStack() as ctx:
        const = ctx.enter_context(tc.tile_pool(name="const", bufs=1))
        wqp = ctx.enter_context(tc.tile_pool(name="wq", bufs=5))
        wop = ctx.enter_context(tc.tile_pool(name="wo", bufs=3))
        ktp = ctx.enter_context(tc.tile_pool(name="kt", bufs=3))
        ktsp = ctx.enter_context(tc.tile_pool(name="kts", bufs=3))
        krotp = ctx.enter_context(tc.tile_pool(name="krot", bufs=3))
        ktmpp = ctx.enter_context(tc.tile_pool(name="ktmp", bufs=2))
        vvp = ctx.enter_context(tc.tile_pool(name="vv", bufs=2))
        expp = ctx.enter_context(tc.tile_pool(name="expp", bufs=2))
        tmp = ctx.enter_context(tc.tile_pool(name="tmp", bufs=6))
        outp = ctx.enter_context(tc.tile_pool(name="outp", bufs=2))
        ps = ctx.enter_context(tc.tile_pool(name="ps", bufs=1, space="PSUM"))

        # one-time constants
        ones_sb = const.tile([128, 1], bf16, tag="ones")
        nc.vector.memset(ones_sb, 1.0)
        onesrow_sb = const.tile([1, 128], bf16, tag="onesrow")
        nc.vector.memset(onesrow_sb, 1.0)
        id_sb = const.tile([128, 128], bf16, tag="ident")
        make_identity(nc, id_sb)

        # PE warm-up: dummy matmuls on the on-chip identity while startup
        # DMAs are in flight (fills the initial PE hole and carries HAM ramp)
        warm_ps = ps.tile([128, 128], f32, tag="qkv", bufs=2)
        for _ in range(24):
            nc.tensor.matmul(warm_ps, id_sb, id_sb, start=True, stop=True)

        def emit_body():
            # ------- startup loads (first weight tiles prefetched) -------
            wq_tiles = {}
            for nch in range(2):
                wq = wqp.tile([128, KO, 128], bf16, tag="wq")
                nc.sync.dma_start(out=wq, in_=wq_d[nch])
                wq_tiles[nch] = wq
            hs_sb = const.tile([128, KO, M], bf16, tag="hs")
            nc.sync.dma_start(out=hs_sb, in_=hsT_d[:])

            cos_sb = const.tile([128, KV], bf16, tag="cos")
            nc.sync.dma_start(out=cos_sb, in_=cos_d[:])
            sins_sb = const.tile([128, KV], bf16, tag="sins")
            nc.sync.dma_start(out=sins_sb, in_=sins_d[:])
            cosq_sb = const.tile([128, B_LOC, Q], bf16, tag="cosq")
            nc.sync.dma_start(out=cosq_sb, in_=cosq_d[:])
            sinq_sb = const.tile([128, B_LOC, Q], bf16, tag="sinq")
            nc.sync.dma_start(out=sinq_sb, in_=sinq_d[:])

            maskT_sb = const.tile([128, B_LOC, Q], f32, tag="maskT")
            nc.sync.dma_start(out=maskT_sb, in_=maskT_d[:])
            qT_sb = const.tile([128, B_LOC, H, Q], bf16, tag="qT")
            knT_sb = const.tile([128, B_LOC, HKV, Q], bf16, tag="knT")
            vn_sb = const.tile([128, B_LOC, HKV, D], bf16, tag="vn")
            attnT_sb = const.tile([128, B_LOC, H, Q], bf16, tag="attnT")
            sums_sb = const.tile([1, NGRP, M], f32, tag="sums")
            rec_all = const.tile([1, NGRP, M], bf16, tag="recall")

            def rope_from_psum(seg, cos_ap, sins_ap, out_ap):
                w = seg.shape[-1]
                tcos = tmp.tile([128, w], bf16, tag="tcos")
                nc.vector.tensor_tensor(out=tcos, in0=seg, in1=cos_ap, op=mybir.AluOpType.mult)
                tsin = tmp.tile([128, w], bf16, tag="tsin")
                nc.vector.tensor_tensor(
                    out=tsin[0:64, :], in0=seg[64:128, :], in1=sins_ap[0:64, :],
                    op=mybir.AluOpType.mult,
                )
                nc.vector.tensor_tensor(
                    out=tsin[64:128, :], in0=seg[0:64, :], in1=sins_ap[64:128, :],
                    op=mybir.AluOpType.mult,
                )
                nc.vector.tensor_tensor(out=out_ap, in0=tcos, in1=tsin, op=mybir.AluOpType.add)

            # ---------- phase 1: qkv^T projection ----------
            for nch in range(NCH):
                hkv, slot = nch // 4, nch % 4
                if nch in wq_tiles:
                    wq = wq_tiles.pop(nch)
                else:
                    wq = wqp.tile([128, KO, 128], bf16, tag="wq")
                    nc.sync.dma_start(out=wq, in_=wq_d[nch])
                pq = ps.tile([128, M], f32, tag="qkv", bufs=2)
                for ko in range(KO):
                    nc.tensor.matmul(
                        pq, wq[:, ko, :], hs_sb[:, ko, :],
                        start=(ko == 0), stop=(ko == KO - 1),
                    )
                if slot <= 1:
                    h = hkv * G + slot
                    for b in range(B_LOC):
                        rope_from_psum(
                            pq[:, b * Q:(b + 1) * Q],
                            cosq_sb[:, b, :], sinq_sb[:, b, :],
                            qT_sb[:, b, h, :],
                        )
                elif slot == 2:
                    for b in range(B_LOC):
                        rope_from_psum(
                            pq[:, b * Q:(b + 1) * Q],
                            cos_sb[:, VLM:VLM + Q], sins_sb[:, VLM:VLM + Q],
                            knT_sb[:, b, hkv, :],
                        )
                else:
                    vt = tmp.tile([128, M], bf16, tag="vt", bufs=3)
                    nc.vector.tensor_copy(out=vt, in_=pq)
                    for b in range(B_LOC):
                        pvt = ps.tile([128, 128], bf16, tag="qkv", bufs=2)
                        nc.tensor.transpose(pvt, vt[:, b * Q:(b + 1) * Q], id_sb)
                        nc.vector.tensor_copy(out=vn_sb[:, b, hkv, :], in_=pvt)

            # ---------- phase 2: attention per (b, hkv) ----------
            for b in range(B_LOC):
                for hkv in range(HKV):
                    grp = b * HKV + hkv
                    kt = ktp.tile([128, VLM], bf16, tag="kt")
                    nc.sync.dma_start(out=kt, in_=vkT_d[b, hkv])
                    kts = ktsp.tile([128, VLM], bf16, tag="kts")
                    nc.sync.dma_start(out=kts, in_=vkTs_d[b, hkv])
                    vvt = vvp.tile([128, KO, D], bf16, tag="vv")
                    nc.sync.dma_start(out=vvt, in_=vv_d[b, hkv])

                    krot = krotp.tile([128, VLM], bf16, tag="krot")
                    nc.vector.tensor_tensor(out=krot, in0=kt, in1=cos_sb[:, 0:VLM], op=mybir.AluOpType.mult)
                    ktmp = ktmpp.tile([128, VLM], bf16, tag="ktmp")
                    nc.vector.tensor_tensor(out=ktmp, in0=kts, in1=sins_sb[:, 0:VLM], op=mybir.AluOpType.mult)
                    nc.vector.tensor_tensor(out=krot, in0=krot, in1=ktmp, op=mybir.AluOpType.add)

                    qT_ap = qT_sb[:, b, hkv * G:(hkv + 1) * G, :]
                    expT = expp.tile([128, KVCH, M], bf16, tag="expT")

                    for cc in range((KVCH + 1) // 2):
                        c0 = cc * 2
                        npair = 2 if c0 + 1 < KVCH else 1
                        pqk = ps.tile([128, 512], f32, tag="qk", bufs=3)
                        for half in range(npair):
                            c = c0 + half
                            lhsT = krot[:, c * 128:(c + 1) * 128] if c < VLM // 128 \
                                else knT_sb[:, b, hkv, :]
                            nc.tensor.matmul(
                                pqk[:, half * M:(half + 1) * M], lhsT, qT_ap,
                                start=True, stop=True,
                            )
                            if c == KVCH - 1:
                                mask_b = maskT_sb[:, b, :]
                                mask_bc = bass.AP(
                                    tensor=mask_b.tensor, offset=mask_b.offset,
                                    ap=[mask_b.ap[0], [0, G], mask_b.ap[1]],
                                )
                                seg = pqk[:, half * M:(half + 1) * M]
                                nc.vector.tensor_tensor(out=seg, in0=seg, in1=mask_bc, op=mybir.AluOpType.add)
                        nc.scalar.activation(
                            out=expT[:, c0:c0 + npair, :], in_=pqk[:, 0:npair * M],
                            func=mybir.ActivationFunctionType.Exp,
                        )

                    po = ps.tile([128, M], f32, tag="pv", bufs=2)
                    psum_s = ps.tile([128, M], f32, tag="sum", bufs=1)
                    for c in range(KVCH):
                        lhsT = vvt[:, c, :] if c < VLM // 128 else vn_sb[:, b, hkv, :]
                        nc.tensor.matmul(po, lhsT, expT[:, c, :], start=(c == 0), stop=(c == KVCH - 1))
                    # exp-sums: 4 col-tiled M=1 accumulations run concurrently in the
                    # PE array; chunk-major order so adjacent MMs hit different col groups
                    for c in range(KVCH):
                        j = c % 4
                        nc.tensor.matmul(
                            psum_s[32 * j:32 * j + 1, :], ones_sb, expT[:, c, :],
                            start=(c < 4), stop=(c >= KVCH - 4),
                            tile_position=(0, 32 * j),
                        )
                    nc.vector.tensor_copy(out=attnT_sb[:, b, hkv * G:(hkv + 1) * G, :], in_=po)
                    scomb = tmp.tile([1, M], f32, tag="scomb", bufs=2)
                    nc.vector.tensor_copy(out=scomb, in_=psum_s[0:1, :])
                    for j in (1, 2):
                        nc.vector.tensor_tensor(out=scomb, in0=psum_s[32 * j:32 * j + 1, :], in1=scomb, op=mybir.AluOpType.add)
                    nc.vector.tensor_tensor(out=sums_sb[:, grp, :], in0=psum_s[96:97, :], in1=scomb, op=mybir.AluOpType.add)

            # ------- softmax normalization (deferred; 2 ACT table loads) ----
            lns = tmp.tile([1, NGRP, M], f32, tag="lns", bufs=1)
            NSL = 4
            for sl in range(NSL):
                g0, g1 = sl * (NGRP // NSL), (sl + 1) * (NGRP // NSL)
                nc.scalar.activation(out=lns[:, g0:g1, :], in_=sums_sb[:, g0:g1, :],
                                     func=mybir.ActivationFunctionType.Ln)
                nc.scalar.activation(out=rec_all[:, g0:g1, :], in_=lns[:, g0:g1, :],
                                     func=mybir.ActivationFunctionType.Exp, scale=-1.0)
            for b in range(B_LOC):
                for hkv in range(HKV):
                    grp = b * HKV + hkv
                    prec = ps.tile([128, M], f32, tag="pv", bufs=2)
                    nc.tensor.matmul(prec, onesrow_sb, rec_all[:, grp, :], start=True, stop=True)
                    rec128 = tmp.tile([128, M], bf16, tag="rec128", bufs=2)
                    nc.vector.tensor_copy(out=rec128, in_=prec)
                    at = attnT_sb[:, b, hkv * G:(hkv + 1) * G, :]
                    nc.vector.tensor_tensor(out=at, in0=at, in1=rec128, op=mybir.AluOpType.mult)

            # ---------- phase 3: output projection ----------
            for nt in range(NWT):
                wo_t = wop.tile([128, H, WOT], bf16, tag="wo")
                nc.sync.dma_start(out=wo_t, in_=wo_d[nt])
                for b in range(B_LOC):
                    pw = ps.tile([128, WOT], f32, tag="qk", bufs=3)
                    for h in range(H):
                        nc.tensor.matmul(
                            pw, attnT_sb[:, b, h, :], wo_t[:, h, :],
                            start=(h == 0), stop=(h == H - 1),
                        )
                    ot = outp.tile([128, WOT], f32, tag="ot")
                    nc.vector.tensor_copy(out=ot, in_=pw)
                    nc.sync.dma_start(out=out_d[b, :, nt * WOT:(nt + 1) * WOT], in_=ot)

        for _rep in range(repeat):
            emit_body()

    nc.finalize()
    return nc


_NC_CACHE = None


def _get_nc():
    global _NC_CACHE
    if _NC_CACHE is None:
        _NC_CACHE = _build_nc()
    return _NC_CACHE


def _host_prep(hidden_states, vlm_key, vlm_value, position_ids, attention_mask,
               wqkv_w, wo_w):
    hs = np.asarray(hidden_states, dtype=np.float32)
    vk = np.asarray(vlm_key, dtype=np.float32)
    vv = np.asarray(vlm_value, dtype=np.float32)
    pos = np.asarray(position_ids).astype(np.int64)
    am = np.asarray(attention_mask, dtype=np.float32)
    wqkv = np.asarray(wqkv_w, dtype=np.float32)
    wo = np.asarray(wo_w, dtype=np.float32)

    # wqkv^T reordered: (nch, ki, ko, nj) fully contiguous per chunk
    wqkvT = wqkv.T.astype(BF)                                  # (2048, 4096)
    wq_r = np.ascontiguousarray(
        wqkvT.reshape(KO, 128, NCH, 128).transpose(2, 1, 0, 3))  # (32,128,16,128)
    # wo^T reordered: (nt, d, h, n)
    woT = wo.T.astype(BF)                                      # (hd, n)
    wo_r = np.ascontiguousarray(
        woT.reshape(H, 128, NWT, WOT).transpose(2, 1, 0, 3))     # (8,128,16,256)

    inv = 1.0 / (THETA ** (np.arange(0, D, 2, dtype=np.float32) / D))
    t = np.arange(KV, dtype=np.float32)
    fr = np.outer(t, inv)
    emb = np.concatenate([fr, fr], axis=-1)
    cosT = np.ascontiguousarray(np.cos(emb).T)       # (D, KV) fp32
    sinT = np.ascontiguousarray(np.sin(emb).T)
    sinTs = sinT.copy()
    sinTs[: D // 2] *= -1.0
    scale = 1.0 / np.sqrt(np.float32(D))

    in_maps = []
    for core in range(N_CORES):
        bs = slice(core * B_LOC, (core + 1) * B_LOC)
        hsT_i = np.ascontiguousarray(
            hs[bs].transpose(2, 0, 1).reshape(KO, 128, M).transpose(1, 0, 2)
        ).astype(BF)                                  # (128, 16, 256)
        vkT_i = np.ascontiguousarray(vk[bs].transpose(0, 1, 3, 2)).astype(BF)
        vkTs_i = np.ascontiguousarray(
            np.concatenate([vkT_i[:, :, D // 2:, :], vkT_i[:, :, : D // 2, :]], axis=2))
        vv_i = np.ascontiguousarray(
            vv[bs].reshape(B_LOC, HKV, KO, 128, D).transpose(0, 1, 3, 2, 4)
        ).astype(BF)                                  # (2,8,128,16,128)
        maskT_i = np.ascontiguousarray(
            np.maximum(am[bs, 0, :, VLM:], -30.0).transpose(2, 0, 1)
        ).astype(np.float32)
        posq = pos[bs] + KV - Q
        cosq_i = np.ascontiguousarray((cosT[:, posq] * scale)).astype(BF)   # (128,2,128)
        sinq_i = np.ascontiguousarray((sinTs[:, posq] * scale)).astype(BF)
        in_maps.append({
            "hsT": hsT_i,
            "wqr": wq_r,
            "wor": wo_r,
            "vkT": vkT_i,
            "vkTs": vkTs_i,
            "vvr": vv_i,
            "maskT": maskT_i,
            "cosT": cosT.astype(BF),
            "sinTs": sinTs.astype(BF),
            "cosqT": cosq_i,
            "sinqTs": sinq_i,
        })
    return in_maps


def kernel(hidden_states, vlm_key, vlm_value, position_ids, attention_mask,
           wqkv_w, wo_w, _trace=False):
    nc = _get_nc()
    in_maps = _host_prep(hidden_states, vlm_key, vlm_value, position_ids,
                         attention_mask, wqkv_w, wo_w)
    res = run_bass_kernel_spmd(nc, in_maps, core_ids=list(range(N_CORES)), trace=_trace)
    out = np.concatenate([res.results[i]["out"] for i in range(N_CORES)], axis=0)
    if _trace:
        kernel._last_results = res
    return out.astype(np.float32)


if __name__ == "__main__":
    rng = np.random.default_rng(0)
    ins = {
        "hidden_states": rng.standard_normal((B, Q, HID), dtype=np.float32),
        "vlm_key": rng.standard_normal((B, HKV, VLM, D), dtype=np.float32),
        "vlm_value": rng.standard_normal((B, HKV, VLM, D), dtype=np.float32),
        "position_ids": np.tile(np.arange(Q, dtype=np.int32), (B, 1)),
        "attention_mask": np.zeros((B, 1, Q, KV), dtype=np.float32),
        "wqkv_w": rng.standard_normal((NQKV, HID), dtype=np.float32) * 0.02,
        "wo_w": rng.standard_normal((HID, HID), dtype=np.float32) * 0.02,
    }
    out = kernel(**ins)
    print("out", out.shape, out.dtype, float(np.abs(out).max()))


# revision 31
# speedup vs baseline: 1.0066x; 1.0058x over previous
"""Trainium2 Bass kernel for ActionExpertAttention (dense transformer block).

Strategy: data-parallel over batch (16 batches -> 2 per core on 8 cores).
All matmuls run in bf16 with fp32 PSUM accumulation. The whole pipeline is
computed in "transposed" space so nothing needs an on-chip transpose except
V_new (16 small PE transposes):

  qkv^T[n, m]   = wqkv^T_chunk^T . hs^T          (n-chunks of 128)
  scores^T[k,q] = Krot^T_chunk^T . Qrot^T        (kv-chunks of 128)
  out^T[d, q]   = V_chunk^T      . exp(scores^T) (accumulated over kv)
  final[q, n]   = attn^T_chunk^T . wo^T          (accumulated over heads)

Softmax denominators come from ones-vector matmuls over exp(scores^T) that
run 4-at-a-time in the PE array via column tiling (tile_position), are
combined with partition-shifted DVE adds, deferred-inverted with sliced
Ln + Exp(-x) passes on ACT (2 activation-table loads total instead of 32),
broadcast across partitions with a tiny K=1 outer-product matmul, and applied
in-place to the unnormalized attn^T. RoPE rotate-half uses sign-baked sin
tables and a host-prepared half-swapped K copy streamed from HBM. All large
DMAs are host-reordered to be fully contiguous.
"""

import sys

sys.path.insert(0, "/opt/trn_rl_repo")

import numpy as np
import ml_dtypes

import concourse.bass as bass
import concourse.tile as tile
from concourse import mybir, bacc
from concourse.bass_utils import run_bass_kernel_spmd
from concourse.masks import make_identity

BF = ml_dtypes.bfloat16

B, Q, VLM = 16, 128, 2048
H, HKV, D = 16, 8, 128
HID = H * D            # 2048
G = H // HKV           # 2
KV = VLM + Q           # 2176
THETA = 10000.0
N_CORES = 8
B_LOC = B // N_CORES   # 2
KDIM = HID
NQKV = (H + 2 * HKV) * D  # 4096
KO = KDIM // 128       # 16
NCH = NQKV // 128      # 32
KVCH = KV // 128       # 17
M = B_LOC * Q          # 256
NGRP = B_LOC * HKV     # 16
WOT = 256              # wo n-tile width
NWT = HID // WOT       # 8

f32 = mybir.dt.float32
bf16 = mybir.dt.bfloat16


def _build_nc(repeat=1):
    nc = bacc.Bacc(trn_type="TRN2", num_swdge_queues=4)

    # all big inputs host-reordered so every DMA is fully contiguous
    hsT_d = nc.dram_tensor("hsT", [128, KO, M], bf16, kind="ExternalInput")
    wq_d = nc.dram_tensor("wqr", [NCH, 128, KO, 128], bf16, kind="ExternalInput")
    wo_d = nc.dram_tensor("wor", [NWT, 128, H, WOT], bf16, kind="ExternalInput")
    vkT_d = nc.dram_tensor("vkT", [B_LOC, HKV, D, VLM], bf16, kind="ExternalInput")
    vkTs_d = nc.dram_tensor("vkTs", [B_LOC, HKV, D, VLM], bf16, kind="ExternalInput")
    vv_d = nc.dram_tensor("vvr", [B_LOC, HKV, 128, KO, D], bf16, kind="ExternalInput")
    maskT_d = nc.dram_tensor("maskT", [Q, B_LOC, Q], f32, kind="ExternalInput")
    sel_d = nc.dram_tensor("sel", [NGRP, NGRP * 128], bf16, kind="ExternalInput")
    cos_d = nc.dram_tensor("cosT", [D, KV], bf16, kind="ExternalInput")
    sins_d = nc.dram_tensor("sinTs", [D, KV], bf16, kind="ExternalInput")
    cosq_d = nc.dram_tensor("cosqT", [D, B_LOC, Q], bf16, kind="ExternalInput")
    sinq_d = nc.dram_tensor("sinqTs", [D, B_LOC, Q], bf16, kind="ExternalInput")
    out_d = nc.dram_tensor("out", [B_LOC, Q, HID], f32, kind="ExternalOutput")

    from contextlib import ExitStack
    with tile.TileContext(nc) as tc, ExitStack() as ctx:
        const = ctx.enter_context(tc.tile_pool(name="const", bufs=1))
        wqp = ctx.enter_context(tc.tile_pool(name="wq", bufs=5))
        wop = ctx.enter_context(tc.tile_pool(name="wo", bufs=3))
        ktp = ctx.enter_context(tc.tile_pool(name="kt", bufs=3))
        ktsp = ctx.enter_context(tc.tile_pool(name="kts", bufs=3))
        krotp = ctx.enter_context(tc.tile_pool(name="krot", bufs=3))
        ktmpp = ctx.enter_context(tc.tile_pool(name="ktmp", bufs=2))
        vvp = ctx.enter_context(tc.tile_pool(name="vv", bufs=2))
        expp = ctx.enter_context(tc.tile_pool(name="expp", bufs=2))
        tmp = ctx.enter_context(tc.tile_pool(name="tmp", bufs=6))
        outp = ctx.enter_context(tc.tile_pool(name="outp", bufs=2))
        ps = ctx.enter_context(tc.tile_pool(name="ps", bufs=1, space="PSUM"))

        # one-time constants
        ones_sb = const.tile([128, 1], bf16, tag="ones")
        nc.vector.memset(ones_sb, 1.0)
        onesrow_sb = const.tile([1, 128], bf16, tag="onesrow")
        nc.vector.memset(onesrow_sb, 1.0)
        id_sb = const.tile([128, 128], bf16, tag="ident")
        make_identity(nc, id_sb)

        # PE warm-up: dummy matmuls on the on-chip identity while startup
        # DMAs are in flight (fills the initial PE hole and carries HAM ramp)
        warm_ps = ps.tile([128, 128], f32, tag="qkv", bufs=2)
        for _ in range(24):
            nc.tensor.matmul(warm_ps, id_sb, id_sb, start=True, stop=True)

        def emit_body():
            # ------- startup loads (first weight tiles prefetched) -------
            wq_tiles = {}
            for nch in range(2):
                wq = wqp.tile([128, KO, 128], bf16, tag="wq")
                nc.sync.dma_start(out=wq, in_=wq_d[nch])
                wq_tiles[nch] = wq
            hs_sb = const.tile([128, KO, M], bf16, tag="hs")
            nc.sync.dma_start(out=hs_sb, in_=hsT_d[:])

            cos_sb = const.tile([128, KV], bf16, tag="cos")
            nc.sync.dma_start(out=cos_sb, in_=cos_d[:])
            sins_sb = const.tile([128, KV], bf16, tag="sins")
            nc.sync.dma_start(out=sins_sb, in_=sins_d[:])
            cosq_sb = const.tile([128, B_LOC, Q], bf16, tag="cosq")
            nc.sync.dma_start(out=cosq_sb, in_=cosq_d[:])
            sinq_sb = const.tile([128, B_LOC, Q], bf16, tag="sinq")
            nc.sync.dma_start(out=sinq_sb, in_=sinq_d[:])

            maskT_sb = const.tile([128, B_LOC, Q], f32, tag="maskT")
            nc.sync.dma_start(out=maskT_sb, in_=maskT_d[:])
            qT_sb = const.tile([128, B_LOC, H, Q], bf16, tag="qT")
            knT_sb = const.tile([128, B_LOC, HKV, Q], bf16, tag="knT")
            vn_sb = const.tile([128, B_LOC, HKV, D], bf16, tag="vn")
            attnT_sb = const.tile([128, B_LOC, H, Q], bf16, tag="attnT")
            sums16_sb = const.tile([NGRP, M], f32, tag="sums16")
            sel_sb = const.tile([NGRP, NGRP, 128], bf16, tag="sel")
            nc.sync.dma_start(out=sel_sb, in_=sel_d.ap().rearrange("p (g m) -> p g m", g=NGRP))

            def rope_from_psum(seg, cos_ap, sins_ap, out_ap):
                w = seg.shape[-1]
                tcos = tmp.tile([128, w], bf16, tag="tcos")
                nc.vector.tensor_tensor(out=tcos, in0=seg, in1=cos_ap, op=mybir.AluOpType.mult)
                tsin = tmp.tile([128, w], bf16, tag="tsin")
                nc.vector.tensor_tensor(
                    out=tsin[0:64, :], in0=seg[64:128, :], in1=sins_ap[0:64, :],
                    op=mybir.AluOpType.mult,
                )
                nc.vector.tensor_tensor(
                    out=tsin[64:128, :], in0=seg[0:64, :], in1=sins_ap[64:128, :],
                    op=mybir.AluOpType.mult,
                )
                nc.vector.tensor_tensor(out=out_ap, in0=tcos, in1=tsin, op=mybir.AluOpType.add)

            # ---------- phase 1: qkv^T projection ----------
            for nch in range(NCH):
                hkv, slot = nch // 4, nch % 4
                if nch in wq_tiles:
                    wq = wq_tiles.pop(nch)
                else:
                    wq = wqp.tile([128, KO, 128], bf16, tag="wq")
                    nc.sync.dma_start(out=wq, in_=wq_d[nch])
                pq = ps.tile([128, M], f32, tag="qkv", bufs=2)
                for ko in range(KO):
                    nc.tensor.matmul(
                        pq, wq[:, ko, :], hs_sb[:, ko, :],
                        start=(ko == 0), stop=(ko == KO - 1),
                    )
                if slot <= 1:
                    h = hkv * G + slot
                    for b in range(B_LOC):
                        rope_from_psum(
                            pq[:, b * Q:(b + 1) * Q],
                            cosq_sb[:, b, :], sinq_sb[:, b, :],
                            qT_sb[:, b, h, :],
                        )
                elif slot == 2:
                    for b in range(B_LOC):
                        rope_from_psum(
                            pq[:, b * Q:(b + 1) * Q],
                            cos_sb[:, VLM:VLM + Q], sins_sb[:, VLM:VLM + Q],
                            knT_sb[:, b, hkv, :],
                        )
                else:
                    vt = tmp.tile([128, M], bf16, tag="vt", bufs=3)
                    nc.vector.tensor_copy(out=vt, in_=pq)
                    for b in range(B_LOC):
                        pvt = ps.tile([128, 128], bf16, tag="qkv", bufs=2)
                        nc.tensor.transpose(pvt, vt[:, b * Q:(b + 1) * Q], id_sb)
                        nc.vector.tensor_copy(out=vn_sb[:, b, hkv, :], in_=pvt)

            # ---------- phase 2: attention per (b, hkv) ----------
            for b in range(B_LOC):
                for hkv in range(HKV):
                    grp = b * HKV + hkv
                    kt = ktp.tile([128, VLM], bf16, tag="kt")
                    nc.sync.dma_start(out=kt, in_=vkT_d[b, hkv])
                    kts = ktsp.tile([128, VLM], bf16, tag="kts")
                    nc.sync.dma_start(out=kts, in_=vkTs_d[b, hkv])
                    vvt = vvp.tile([128, KO, D], bf16, tag="vv")
                    nc.sync.dma_start(out=vvt, in_=vv_d[b, hkv])

                    krot = krotp.tile([128, VLM], bf16, tag="krot")
                    nc.vector.tensor_tensor(out=krot, in0=kt, in1=cos_sb[:, 0:VLM], op=mybir.AluOpType.mult)
                    ktmp = ktmpp.tile([128, VLM], bf16, tag="ktmp")
                    nc.vector.tensor_tensor(out=ktmp, in0=kts, in1=sins_sb[:, 0:VLM], op=mybir.AluOpType.mult)
                    nc.vector.tensor_tensor(out=krot, in0=krot, in1=ktmp, op=mybir.AluOpType.add)

                    qT_ap = qT_sb[:, b, hkv * G:(hkv + 1) * G, :]
                    expT = expp.tile([128, KVCH, M], bf16, tag="expT")

                    for cc in range((KVCH + 1) // 2):
                        c0 = cc * 2
                        npair = 2 if c0 + 1 < KVCH else 1
                        pqk = ps.tile([128, 512], f32, tag="qk", bufs=3)
                        for half in range(npair):
                            c = c0 + half
                            lhsT = krot[:, c * 128:(c + 1) * 128] if c < VLM // 128 \
                                else knT_sb[:, b, hkv, :]
                            nc.tensor.matmul(
                                pqk[:, half * M:(half + 1) * M], lhsT, qT_ap,
                                start=True, stop=True,
                            )
                            if c == KVCH - 1:
                                mask_b = maskT_sb[:, b, :]
                                mask_bc = bass.AP(
                                    tensor=mask_b.tensor, offset=mask_b.offset,
                                    ap=[mask_b.ap[0], [0, G], mask_b.ap[1]],
                                )
                                seg = pqk[:, half * M:(half + 1) * M]
                                nc.vector.tensor_tensor(out=seg, in0=seg, in1=mask_bc, op=mybir.AluOpType.add)
                        nc.scalar.activation(
                            out=expT[:, c0:c0 + npair, :], in_=pqk[:, 0:npair * M],
                            func=mybir.ActivationFunctionType.Exp,
                        )

                    po = ps.tile([128, M], f32, tag="pv", bufs=2)
                    psum_s = ps.tile([128, M], f32, tag="sum", bufs=1)
                    for c in range(KVCH):
                        lhsT = vvt[:, c, :] if c < VLM // 128 else vn_sb[:, b, hkv, :]
                        nc.tensor.matmul(po, lhsT, expT[:, c, :], start=(c == 0), stop=(c == KVCH - 1))
                    # exp-sums: 4 col-tiled M=1 accumulations run concurrently in the
                    # PE array; chunk-major order so adjacent MMs hit different col groups
                    for c in range(KVCH):
                        j = c % 4
                        nc.tensor.matmul(
                            psum_s[32 * j:32 * j + 1, :], ones_sb, expT[:, c, :],
                            start=(c < 4), stop=(c >= KVCH - 4),
                            tile_position=(0, 32 * j),
                        )
                    nc.vector.tensor_copy(out=attnT_sb[:, b, hkv * G:(hkv + 1) * G, :], in_=po)
                    scomb = tmp.tile([1, M], f32, tag="scomb", bufs=2)
                    nc.vector.tensor_copy(out=scomb, in_=psum_s[0:1, :])
                    for j in (1, 2):
                        nc.vector.tensor_tensor(out=scomb, in0=psum_s[32 * j:32 * j + 1, :], in1=scomb, op=mybir.AluOpType.add)
                    scomb2 = tmp.tile([1, M], f32, tag="scomb2", bufs=2)
                    nc.vector.tensor_tensor(out=scomb2, in0=psum_s[96:97, :], in1=scomb, op=mybir.AluOpType.add)
                    nc.sync.dma_start(out=sums16_sb[grp:grp + 1, :], in_=scomb2)

            # ------- softmax normalization: one parallel DVE reciprocal ------
            rec16 = tmp.tile([NGRP, M], f32, tag="rec16", bufs=1)
            nc.vector.reciprocal(out=rec16, in_=sums16_sb)
            rec16b = tmp.tile([NGRP, M], bf16, tag="rec16b", bufs=1)
            nc.vector.tensor_copy(out=rec16b, in_=rec16)
            for b in range(B_LOC):
                for hkv in range(HKV):
                    grp = b * HKV + hkv
                    prec = ps.tile([128, M], f32, tag="pv", bufs=2)
                    nc.tensor.matmul(prec, sel_sb[:, grp, :], rec16b, start=True, stop=True)
                    rec128 = tmp.tile([128, M], bf16, tag="rec128", bufs=2)
                    nc.vector.tensor_copy(out=rec128, in_=prec)
                    at = attnT_sb[:, b, hkv * G:(hkv + 1) * G, :]
                    nc.vector.tensor_tensor(out=at, in0=at, in1=rec128, op=mybir.AluOpType.mult)

            # ---------- phase 3: output projection ----------
            for nt in range(NWT):
                wo_t = wop.tile([128, H, WOT], bf16, tag="wo")
                nc.sync.dma_start(out=wo_t, in_=wo_d[nt])
                for b in range(B_LOC):
                    pw = ps.tile([128, WOT], f32, tag="qk", bufs=3)
                    for h in range(H):
                        nc.tensor.matmul(
                            pw, attnT_sb[:, b, h, :], wo_t[:, h, :],
                            start=(h == 0), stop=(h == H - 1),
                        )
                    ot = outp.tile([128, WOT], f32, tag="ot")
                    nc.vector.tensor_copy(out=ot, in_=pw)
                    nc.sync.dma_start(out=out_d[b, :, nt * WOT:(nt + 1) * WOT], in_=ot)

        for _rep in range(repeat):
            emit_body()

    nc.finalize()
    return nc


_NC_CACHE = None


def _get_nc():
    global _NC_CACHE
    if _NC_CACHE is None:
        _NC_CACHE = _build_nc()
    return _NC_CACHE


def _host_prep(hidden_states, vlm_key, vlm_value, position_ids, attention_mask,
               wqkv_w, wo_w):
    hs = np.asarray(hidden_states, dtype=np.float32)
    vk = np.asarray(vlm_key, dtype=np.float32)
    vv = np.asarray(vlm_value, dtype=np.float32)
    pos = np.asarray(position_ids).astype(np.int64)
    am = np.asarray(attention_mask, dtype=np.float32)
    wqkv = np.asarray(wqkv_w, dtype=np.float32)
    wo = np.asarray(wo_w, dtype=np.float32)

    # wqkv^T reordered: (nch, ki, ko, nj) fully contiguous per chunk
    wqkvT = wqkv.T.astype(BF)                                  # (2048, 4096)
    wq_r = np.ascontiguousarray(
        wqkvT.reshape(KO, 128, NCH, 128).transpose(2, 1, 0, 3))  # (32,128,16,128)
    # wo^T reordered: (nt, d, h, n)
    woT = wo.T.astype(BF)                                      # (hd, n)
    wo_r = np.ascontiguousarray(
        woT.reshape(H, 128, NWT, WOT).transpose(2, 1, 0, 3))     # (8,128,16,256)

    inv = 1.0 / (THETA ** (np.arange(0, D, 2, dtype=np.float32) / D))
    t = np.arange(KV, dtype=np.float32)
    fr = np.outer(t, inv)
    emb = np.concatenate([fr, fr], axis=-1)
    cosT = np.ascontiguousarray(np.cos(emb).T)       # (D, KV) fp32
    sinT = np.ascontiguousarray(np.sin(emb).T)
    sinTs = sinT.copy()
    sinTs[: D // 2] *= -1.0
    scale = 1.0 / np.sqrt(np.float32(D))

    sel = np.zeros((NGRP, NGRP * 128), dtype=np.float32)
    for g in range(NGRP):
        sel[g, g * 128:(g + 1) * 128] = 1.0
    sel = sel.astype(BF)

    in_maps = []
    for core in range(N_CORES):
        bs = slice(core * B_LOC, (core + 1) * B_LOC)
        hsT_i = np.ascontiguousarray(
            hs[bs].transpose(2, 0, 1).reshape(KO, 128, M).transpose(1, 0, 2)
        ).astype(BF)                                  # (128, 16, 256)
        vkT_i = np.ascontiguousarray(vk[bs].transpose(0, 1, 3, 2)).astype(BF)
        vkTs_i = np.ascontiguousarray(
            np.concatenate([vkT_i[:, :, D // 2:, :], vkT_i[:, :, : D // 2, :]], axis=2))
        vv_i = np.ascontiguousarray(
            vv[bs].reshape(B_LOC, HKV, KO, 128, D).transpose(0, 1, 3, 2, 4)
        ).astype(BF)                                  # (2,8,128,16,128)
        maskT_i = np.ascontiguousarray(
            np.maximum(am[bs, 0, :, VLM:], -30.0).transpose(2, 0, 1)
        ).astype(np.float32)
        posq = pos[bs] + KV - Q
        cosq_i = np.ascontiguousarray((cosT[:, posq] * scale)).astype(BF)   # (128,2,128)
        sinq_i = np.ascontiguousarray((sinTs[:, posq] * scale)).astype(BF)
        in_maps.append({
            "hsT": hsT_i,
            "wqr": wq_r,
            "wor": wo_r,
            "vkT": vkT_i,
            "vkTs": vkTs_i,
            "vvr": vv_i,
            "maskT": maskT_i,
            "sel": sel,
            "cosT": cosT.astype(BF),
            "sinTs": sinTs.astype(BF),
            "cosqT": cosq_i,
            "sinqTs": sinq_i,
        })
    return in_maps


def kernel(hidden_states, vlm_key, vlm_value, position_ids, attention_mask,
           wqkv_w, wo_w, _trace=False):
    nc = _get_nc()
    in_maps = _host_prep(hidden_states, vlm_key, vlm_value, position_ids,
                         attention_mask, wqkv_w, wo_w)
    res = run_bass_kernel_spmd(nc, in_maps, core_ids=list(range(N_CORES)), trace=_trace)
    out = np.concatenate([res.results[i]["out"] for i in range(N_CORES)], axis=0)
    if _trace:
        kernel._last_results = res
    return out.astype(np.float32)


if __name__ == "__main__":
    rng = np.random.default_rng(0)
    ins = {
        "hidden_states": rng.standard_normal((B, Q, HID), dtype=np.float32),
        "vlm_key": rng.standard_normal((B, HKV, VLM, D), dtype=np.float32),
        "vlm_value": rng.standard_normal((B, HKV, VLM, D), dtype=np.float32),
        "position_ids": np.tile(np.arange(Q, dtype=np.int32), (B, 1)),
        "attention_mask": np.zeros((B, 1, Q, KV), dtype=np.float32),
        "wqkv_w": rng.standard_normal((NQKV, HID), dtype=np.float32) * 0.02,
        "wo_w": rng.standard_normal((HID, HID), dtype=np.float32) * 0.02,
    }
    out = kernel(**ins)
    print("out", out.shape, out.dtype, float(np.abs(out).max()))


# revision 32
# speedup vs baseline: 1.0230x; 1.0163x over previous
"""Trainium2 Bass kernel for ActionExpertAttention (dense transformer block).

Strategy: data-parallel over batch (16 batches -> 2 per core on 8 cores).
All matmuls run in bf16 with fp32 PSUM accumulation. The whole pipeline is
computed in "transposed" space so nothing needs an on-chip transpose except
V_new (16 small PE transposes):

  qkv^T[n, m]   = wqkv^T_chunk^T . hs^T          (n-chunks of 128)
  scores^T[k,q] = Krot^T_chunk^T . Qrot^T        (kv-chunks of 128)
  out^T[d, q]   = V_chunk^T      . exp(scores^T) (accumulated over kv)
  final[q, n]   = attn^T_chunk^T . wo^T          (accumulated over heads)

Softmax denominators come from ones-vector matmuls over exp(scores^T) that
run 4-at-a-time in the PE array via column tiling (tile_position), are
combined with partition-shifted DVE adds, deferred-inverted with sliced
Ln + Exp(-x) passes on ACT (2 activation-table loads total instead of 32),
broadcast across partitions with a tiny K=1 outer-product matmul, and applied
in-place to the unnormalized attn^T. RoPE rotate-half uses sign-baked sin
tables and a host-prepared half-swapped K copy streamed from HBM. All large
DMAs are host-reordered to be fully contiguous.
"""

import sys

sys.path.insert(0, "/opt/trn_rl_repo")

import numpy as np
import ml_dtypes

import concourse.bass as bass
import concourse.tile as tile
from concourse import mybir, bacc
from concourse.bass_utils import run_bass_kernel_spmd
from concourse.masks import make_identity

BF = ml_dtypes.bfloat16

B, Q, VLM = 16, 128, 2048
H, HKV, D = 16, 8, 128
HID = H * D            # 2048
G = H // HKV           # 2
KV = VLM + Q           # 2176
THETA = 10000.0
N_CORES = 8
B_LOC = B // N_CORES   # 2
KDIM = HID
NQKV = (H + 2 * HKV) * D  # 4096
KO = KDIM // 128       # 16
NCH = NQKV // 128      # 32
KVCH = KV // 128       # 17
M = B_LOC * Q          # 256
NGRP = B_LOC * HKV     # 16
WOT = 256              # wo n-tile width
NWT = HID // WOT       # 8

f32 = mybir.dt.float32
bf16 = mybir.dt.bfloat16


def _build_nc(repeat=1):
    nc = bacc.Bacc(trn_type="TRN2", num_swdge_queues=4)

    # all big inputs host-reordered so every DMA is fully contiguous
    hsT_d = nc.dram_tensor("hsT", [128, KO, M], bf16, kind="ExternalInput")
    wq_d = nc.dram_tensor("wqr", [NCH, 128, KO, 128], bf16, kind="ExternalInput")
    wo_d = nc.dram_tensor("wor", [NWT, 128, H, WOT], bf16, kind="ExternalInput")
    vkT_d = nc.dram_tensor("vkT", [B_LOC, HKV, D, VLM], bf16, kind="ExternalInput")
    vkTs_d = nc.dram_tensor("vkTs", [B_LOC, HKV, D, VLM], bf16, kind="ExternalInput")
    vv_d = nc.dram_tensor("vvr", [B_LOC, HKV, 128, KO, D], bf16, kind="ExternalInput")
    maskT_d = nc.dram_tensor("maskT", [Q, B_LOC, Q], f32, kind="ExternalInput")
    sel_d = nc.dram_tensor("sel", [NGRP, NGRP * 128], bf16, kind="ExternalInput")
    cos_d = nc.dram_tensor("cosT", [D, KV], bf16, kind="ExternalInput")
    sins_d = nc.dram_tensor("sinTs", [D, KV], bf16, kind="ExternalInput")
    cosq_d = nc.dram_tensor("cosqT", [D, B_LOC, Q], bf16, kind="ExternalInput")
    sinq_d = nc.dram_tensor("sinqTs", [D, B_LOC, Q], bf16, kind="ExternalInput")
    out_d = nc.dram_tensor("out", [B_LOC, Q, HID], f32, kind="ExternalOutput")

    from contextlib import ExitStack
    with tile.TileContext(nc) as tc, ExitStack() as ctx:
        const = ctx.enter_context(tc.tile_pool(name="const", bufs=1))
        wqp = ctx.enter_context(tc.tile_pool(name="wq", bufs=5))
        wop = ctx.enter_context(tc.tile_pool(name="wo", bufs=3))
        ktp = ctx.enter_context(tc.tile_pool(name="kt", bufs=3))
        ktsp = ctx.enter_context(tc.tile_pool(name="kts", bufs=3))
        krotp = ctx.enter_context(tc.tile_pool(name="krot", bufs=3))
        ktmpp = ctx.enter_context(tc.tile_pool(name="ktmp", bufs=2))
        vvp = ctx.enter_context(tc.tile_pool(name="vv", bufs=2))
        expp = ctx.enter_context(tc.tile_pool(name="expp", bufs=2))
        tmp = ctx.enter_context(tc.tile_pool(name="tmp", bufs=6))
        outp = ctx.enter_context(tc.tile_pool(name="outp", bufs=2))
        ps = ctx.enter_context(tc.tile_pool(name="ps", bufs=1, space="PSUM"))

        # one-time constants
        ones_sb = const.tile([128, 1], bf16, tag="ones")
        nc.vector.memset(ones_sb, 1.0)
        onesrow_sb = const.tile([1, 128], bf16, tag="onesrow")
        nc.vector.memset(onesrow_sb, 1.0)
        id_sb = const.tile([128, 128], bf16, tag="ident")
        make_identity(nc, id_sb)

        # PE warm-up: dummy matmuls on the on-chip identity while startup
        # DMAs are in flight (fills the initial PE hole and carries HAM ramp)
        warm_ps = ps.tile([128, 128], f32, tag="qkv", bufs=2)
        for _ in range(24):
            nc.tensor.matmul(warm_ps, id_sb, id_sb, start=True, stop=True)

        def emit_body():
            # ------- startup loads (first weight tiles prefetched) -------
            wq_tiles = {}
            for nch in range(2):
                wq = wqp.tile([128, KO, 128], bf16, tag="wq")
                nc.sync.dma_start(out=wq, in_=wq_d[nch])
                wq_tiles[nch] = wq
            hs_sb = const.tile([128, KO, M], bf16, tag="hs")
            nc.sync.dma_start(out=hs_sb, in_=hsT_d[:])

            cos_sb = const.tile([128, KV], bf16, tag="cos")
            nc.sync.dma_start(out=cos_sb, in_=cos_d[:])
            sins_sb = const.tile([128, KV], bf16, tag="sins")
            nc.sync.dma_start(out=sins_sb, in_=sins_d[:])
            cosq_sb = const.tile([128, B_LOC, Q], bf16, tag="cosq")
            nc.sync.dma_start(out=cosq_sb, in_=cosq_d[:])
            sinq_sb = const.tile([128, B_LOC, Q], bf16, tag="sinq")
            nc.sync.dma_start(out=sinq_sb, in_=sinq_d[:])

            maskT_sb = const.tile([128, B_LOC, Q], f32, tag="maskT")
            nc.sync.dma_start(out=maskT_sb, in_=maskT_d[:])
            qT_sb = const.tile([128, B_LOC, H, Q], bf16, tag="qT")
            knT_sb = const.tile([128, B_LOC, HKV, Q], bf16, tag="knT")
            vn_sb = const.tile([128, B_LOC, HKV, D], bf16, tag="vn")
            attnT_sb = const.tile([128, B_LOC, H, Q], bf16, tag="attnT")
            sums16_sb = const.tile([NGRP, M], f32, tag="sums16")
            sel_sb = const.tile([NGRP, NGRP, 128], bf16, tag="sel")
            nc.sync.dma_start(out=sel_sb, in_=sel_d.ap().rearrange("p (g m) -> p g m", g=NGRP))

            def rope_from_psum(seg, cos_ap, sins_ap, out_ap):
                w = seg.shape[-1]
                tcos = tmp.tile([128, w], bf16, tag="tcos")
                nc.vector.tensor_tensor(out=tcos, in0=seg, in1=cos_ap, op=mybir.AluOpType.mult)
                tsin = tmp.tile([128, w], bf16, tag="tsin")
                nc.vector.tensor_tensor(
                    out=tsin[0:64, :], in0=seg[64:128, :], in1=sins_ap[0:64, :],
                    op=mybir.AluOpType.mult,
                )
                nc.vector.tensor_tensor(
                    out=tsin[64:128, :], in0=seg[0:64, :], in1=sins_ap[64:128, :],
                    op=mybir.AluOpType.mult,
                )
                nc.vector.tensor_tensor(out=out_ap, in0=tcos, in1=tsin, op=mybir.AluOpType.add)

            # ---------- phase 1: qkv^T projection ----------
            for nch in range(NCH):
                hkv, slot = nch // 4, nch % 4
                if nch in wq_tiles:
                    wq = wq_tiles.pop(nch)
                else:
                    wq = wqp.tile([128, KO, 128], bf16, tag="wq")
                    nc.sync.dma_start(out=wq, in_=wq_d[nch])
                pq = ps.tile([128, M], f32, tag="qkv", bufs=2)
                for ko in range(KO):
                    nc.tensor.matmul(
                        pq, wq[:, ko, :], hs_sb[:, ko, :],
                        start=(ko == 0), stop=(ko == KO - 1),
                    )
                if slot <= 1:
                    h = hkv * G + slot
                    for b in range(B_LOC):
                        rope_from_psum(
                            pq[:, b * Q:(b + 1) * Q],
                            cosq_sb[:, b, :], sinq_sb[:, b, :],
                            qT_sb[:, b, h, :],
                        )
                elif slot == 2:
                    for b in range(B_LOC):
                        rope_from_psum(
                            pq[:, b * Q:(b + 1) * Q],
                            cos_sb[:, VLM:VLM + Q], sins_sb[:, VLM:VLM + Q],
                            knT_sb[:, b, hkv, :],
                        )
                else:
                    vt = tmp.tile([128, M], bf16, tag="vt", bufs=3)
                    nc.vector.tensor_copy(out=vt, in_=pq)
                    for b in range(B_LOC):
                        pvt = ps.tile([128, 128], bf16, tag="qkv", bufs=2)
                        nc.tensor.transpose(pvt, vt[:, b * Q:(b + 1) * Q], id_sb)
                        nc.vector.tensor_copy(out=vn_sb[:, b, hkv, :], in_=pvt)

            # ---------- phase 2: attention per (b, hkv) ----------
            for b in range(B_LOC):
                for hkv in range(HKV):
                    grp = b * HKV + hkv
                    kt = ktp.tile([128, VLM], bf16, tag="kt")
                    nc.sync.dma_start(out=kt, in_=vkT_d[b, hkv])
                    kts = ktsp.tile([128, VLM], bf16, tag="kts")
                    nc.sync.dma_start(out=kts, in_=vkTs_d[b, hkv])
                    vvt = vvp.tile([128, KO, D], bf16, tag="vv")
                    nc.sync.dma_start(out=vvt, in_=vv_d[b, hkv])

                    krot = krotp.tile([128, VLM], bf16, tag="krot")
                    nc.vector.tensor_tensor(out=krot, in0=kt, in1=cos_sb[:, 0:VLM], op=mybir.AluOpType.mult)
                    ktmp = ktmpp.tile([128, VLM], bf16, tag="ktmp")
                    nc.vector.tensor_tensor(out=ktmp, in0=kts, in1=sins_sb[:, 0:VLM], op=mybir.AluOpType.mult)
                    nc.vector.tensor_tensor(out=krot, in0=krot, in1=ktmp, op=mybir.AluOpType.add)

                    qT_ap = qT_sb[:, b, hkv * G:(hkv + 1) * G, :]
                    expT = expp.tile([128, KVCH, M], bf16, tag="expT")

                    for cc in range((KVCH + 1) // 2):
                        c0 = cc * 2
                        npair = 2 if c0 + 1 < KVCH else 1
                        pqk = ps.tile([128, 512], f32, tag="qk", bufs=3)
                        for half in range(npair):
                            c = c0 + half
                            lhsT = krot[:, c * 128:(c + 1) * 128] if c < VLM // 128 \
                                else knT_sb[:, b, hkv, :]
                            nc.tensor.matmul(
                                pqk[:, half * M:(half + 1) * M], lhsT, qT_ap,
                                start=True, stop=True,
                            )
                            if c == KVCH - 1:
                                mask_b = maskT_sb[:, b, :]
                                mask_bc = bass.AP(
                                    tensor=mask_b.tensor, offset=mask_b.offset,
                                    ap=[mask_b.ap[0], [0, G], mask_b.ap[1]],
                                )
                                seg = pqk[:, half * M:(half + 1) * M]
                                nc.vector.tensor_tensor(out=seg, in0=seg, in1=mask_bc, op=mybir.AluOpType.add)
                        nc.scalar.activation(
                            out=expT[:, c0:c0 + npair, :], in_=pqk[:, 0:npair * M],
                            func=mybir.ActivationFunctionType.Exp,
                        )

                    po = ps.tile([128, M], f32, tag="pv", bufs=2)
                    psum_s = ps.tile([128, M], f32, tag="sum", bufs=1)
                    for c in range(KVCH):
                        lhsT = vvt[:, c, :] if c < VLM // 128 else vn_sb[:, b, hkv, :]
                        nc.tensor.matmul(po, lhsT, expT[:, c, :], start=(c == 0), stop=(c == KVCH - 1))
                    # exp-sums: 4 col-tiled M=1 accumulations run concurrently in the
                    # PE array; chunk-major order so adjacent MMs hit different col groups
                    for c in range(KVCH):
                        j = c % 4
                        nc.tensor.matmul(
                            psum_s[32 * j:32 * j + 1, :], ones_sb, expT[:, c, :],
                            start=(c < 4), stop=(c >= KVCH - 4),
                            tile_position=(0, 32 * j),
                        )
                    nc.vector.tensor_copy(out=attnT_sb[:, b, hkv * G:(hkv + 1) * G, :], in_=po)
                    scomb = tmp.tile([1, M], f32, tag="scomb", bufs=2)
                    nc.vector.tensor_copy(out=scomb, in_=psum_s[0:1, :])
                    for j in (1, 2):
                        nc.vector.tensor_tensor(out=scomb, in0=psum_s[32 * j:32 * j + 1, :], in1=scomb, op=mybir.AluOpType.add)
                    scomb2 = tmp.tile([1, M], f32, tag="scomb2", bufs=2)
                    nc.vector.tensor_tensor(out=scomb2, in0=psum_s[96:97, :], in1=scomb, op=mybir.AluOpType.add)
                    nc.sync.dma_start(out=sums16_sb[grp:grp + 1, :], in_=scomb2)

            # ------- softmax normalization: one parallel DVE reciprocal ------
            rec16 = tmp.tile([NGRP, M], f32, tag="rec16", bufs=1)
            nc.vector.reciprocal(out=rec16, in_=sums16_sb)
            rec16b = tmp.tile([NGRP, M], bf16, tag="rec16b", bufs=1)
            nc.vector.tensor_copy(out=rec16b, in_=rec16)
            for b in range(B_LOC):
                for hkv in range(HKV):
                    grp = b * HKV + hkv
                    prec = ps.tile([128, M], f32, tag="pv", bufs=2)
                    nc.tensor.matmul(prec, sel_sb[:, grp, :], rec16b, start=True, stop=True)
                    at = attnT_sb[:, b, hkv * G:(hkv + 1) * G, :]
                    nc.vector.tensor_tensor(out=at, in0=at, in1=prec, op=mybir.AluOpType.mult)

            # ---------- phase 3: output projection ----------
            for nt in range(NWT):
                wo_t = wop.tile([128, H, WOT], bf16, tag="wo")
                nc.sync.dma_start(out=wo_t, in_=wo_d[nt])
                for b in range(B_LOC):
                    pw = ps.tile([128, WOT], f32, tag="qk", bufs=3)
                    for h in range(H):
                        nc.tensor.matmul(
                            pw, attnT_sb[:, b, h, :], wo_t[:, h, :],
                            start=(h == 0), stop=(h == H - 1),
                        )
                    ot = outp.tile([128, WOT], f32, tag="ot")
                    nc.vector.tensor_copy(out=ot, in_=pw)
                    nc.sync.dma_start(out=out_d[b, :, nt * WOT:(nt + 1) * WOT], in_=ot)

        for _rep in range(repeat):
            emit_body()

    nc.finalize()
    return nc


_NC_CACHE = None


def _get_nc():
    global _NC_CACHE
    if _NC_CACHE is None:
        _NC_CACHE = _build_nc()
    return _NC_CACHE


def _host_prep(hidden_states, vlm_key, vlm_value, position_ids, attention_mask,
               wqkv_w, wo_w):
    hs = np.asarray(hidden_states, dtype=np.float32)
    vk = np.asarray(vlm_key, dtype=np.float32)
    vv = np.asarray(vlm_value, dtype=np.float32)
    pos = np.asarray(position_ids).astype(np.int64)
    am = np.asarray(attention_mask, dtype=np.float32)
    wqkv = np.asarray(wqkv_w, dtype=np.float32)
    wo = np.asarray(wo_w, dtype=np.float32)

    # wqkv^T reordered: (nch, ki, ko, nj) fully contiguous per chunk
    wqkvT = wqkv.T.astype(BF)                                  # (2048, 4096)
    wq_r = np.ascontiguousarray(
        wqkvT.reshape(KO, 128, NCH, 128).transpose(2, 1, 0, 3))  # (32,128,16,128)
    # wo^T reordered: (nt, d, h, n)
    woT = wo.T.astype(BF)                                      # (hd, n)
    wo_r = np.ascontiguousarray(
        woT.reshape(H, 128, NWT, WOT).transpose(2, 1, 0, 3))     # (8,128,16,256)

    inv = 1.0 / (THETA ** (np.arange(0, D, 2, dtype=np.float32) / D))
    t = np.arange(KV, dtype=np.float32)
    fr = np.outer(t, inv)
    emb = np.concatenate([fr, fr], axis=-1)
    cosT = np.ascontiguousarray(np.cos(emb).T)       # (D, KV) fp32
    sinT = np.ascontiguousarray(np.sin(emb).T)
    sinTs = sinT.copy()
    sinTs[: D // 2] *= -1.0
    scale = 1.0 / np.sqrt(np.float32(D))

    sel = np.zeros((NGRP, NGRP * 128), dtype=np.float32)
    for g in range(NGRP):
        sel[g, g * 128:(g + 1) * 128] = 1.0
    sel = sel.astype(BF)

    in_maps = []
    for core in range(N_CORES):
        bs = slice(core * B_LOC, (core + 1) * B_LOC)
        hsT_i = np.ascontiguousarray(
            hs[bs].transpose(2, 0, 1).reshape(KO, 128, M).transpose(1, 0, 2)
        ).astype(BF)                                  # (128, 16, 256)
        vkT_i = np.ascontiguousarray(vk[bs].transpose(0, 1, 3, 2)).astype(BF)
        vkTs_i = np.ascontiguousarray(
            np.concatenate([vkT_i[:, :, D // 2:, :], vkT_i[:, :, : D // 2, :]], axis=2))
        vv_i = np.ascontiguousarray(
            vv[bs].reshape(B_LOC, HKV, KO, 128, D).transpose(0, 1, 3, 2, 4)
        ).astype(BF)                                  # (2,8,128,16,128)
        maskT_i = np.ascontiguousarray(
            np.maximum(am[bs, 0, :, VLM:], -30.0).transpose(2, 0, 1)
        ).astype(np.float32)
        posq = pos[bs] + KV - Q
        cosq_i = np.ascontiguousarray((cosT[:, posq] * scale)).astype(BF)   # (128,2,128)
        sinq_i = np.ascontiguousarray((sinTs[:, posq] * scale)).astype(BF)
        in_maps.append({
            "hsT": hsT_i,
            "wqr": wq_r,
            "wor": wo_r,
            "vkT": vkT_i,
            "vkTs": vkTs_i,
            "vvr": vv_i,
            "maskT": maskT_i,
            "sel": sel,
            "cosT": cosT.astype(BF),
            "sinTs": sinTs.astype(BF),
            "cosqT": cosq_i,
            "sinqTs": sinq_i,
        })
    return in_maps


def kernel(hidden_states, vlm_key, vlm_value, position_ids, attention_mask,
           wqkv_w, wo_w, _trace=False):
    nc = _get_nc()
    in_maps = _host_prep(hidden_states, vlm_key, vlm_value, position_ids,
                         attention_mask, wqkv_w, wo_w)
    res = run_bass_kernel_spmd(nc, in_maps, core_ids=list(range(N_CORES)), trace=_trace)
    out = np.concatenate([res.results[i]["out"] for i in range(N_CORES)], axis=0)
    if _trace:
        kernel._last_results = res
    return out.astype(np.float32)


if __name__ == "__main__":
    rng = np.random.default_rng(0)
    ins = {
        "hidden_states": rng.standard_normal((B, Q, HID), dtype=np.float32),
        "vlm_key": rng.standard_normal((B, HKV, VLM, D), dtype=np.float32),
        "vlm_value": rng.standard_normal((B, HKV, VLM, D), dtype=np.float32),
        "position_ids": np.tile(np.arange(Q, dtype=np.int32), (B, 1)),
        "attention_mask": np.zeros((B, 1, Q, KV), dtype=np.float32),
        "wqkv_w": rng.standard_normal((NQKV, HID), dtype=np.float32) * 0.02,
        "wo_w": rng.standard_normal((HID, HID), dtype=np.float32) * 0.02,
    }
    out = kernel(**ins)
    print("out", out.shape, out.dtype, float(np.abs(out).max()))


# revision 33
# speedup vs baseline: 1.0523x; 1.0287x over previous
"""Trainium2 Bass kernel for ActionExpertAttention (dense transformer block).

Strategy: data-parallel over batch (16 batches -> 2 per core on 8 cores).
All matmuls run in bf16 with fp32 PSUM accumulation. The whole pipeline is
computed in "transposed" space so nothing needs an on-chip transpose except
V_new (16 small PE transposes):

  qkv^T[n, m]   = wqkv^T_chunk^T . hs^T          (n-chunks of 128)
  scores^T[k,q] = Krot^T_chunk^T . Qrot^T        (kv-chunks of 128)
  out^T[d, q]   = V_chunk^T      . exp(scores^T) (accumulated over kv)
  final[q, n]   = attn^T_chunk^T . wo^T          (accumulated over heads)

Softmax denominators come from ones-vector matmuls over exp(scores^T) that
run 4-at-a-time in the PE array via column tiling (tile_position), are
combined with partition-shifted DVE adds, deferred-inverted with sliced
Ln + Exp(-x) passes on ACT (2 activation-table loads total instead of 32),
broadcast across partitions with a tiny K=1 outer-product matmul, and applied
in-place to the unnormalized attn^T. RoPE rotate-half uses sign-baked sin
tables and a host-prepared half-swapped K copy streamed from HBM. All large
DMAs are host-reordered to be fully contiguous.
"""

import sys

sys.path.insert(0, "/opt/trn_rl_repo")

import numpy as np
import ml_dtypes

import concourse.bass as bass
import concourse.tile as tile
from concourse import mybir, bacc
from concourse.bass_utils import run_bass_kernel_spmd
from concourse.masks import make_identity

BF = ml_dtypes.bfloat16

B, Q, VLM = 16, 128, 2048
H, HKV, D = 16, 8, 128
HID = H * D            # 2048
G = H // HKV           # 2
KV = VLM + Q           # 2176
THETA = 10000.0
N_CORES = 8
B_LOC = B // N_CORES   # 2
KDIM = HID
NQKV = (H + 2 * HKV) * D  # 4096
KO = KDIM // 128       # 16
NCH = NQKV // 128      # 32
KVCH = KV // 128       # 17
M = B_LOC * Q          # 256
NGRP = B_LOC * HKV     # 16
WOT = 256              # wo n-tile width
NWT = HID // WOT       # 8

f32 = mybir.dt.float32
bf16 = mybir.dt.bfloat16


def _build_nc(repeat=1):
    nc = bacc.Bacc(trn_type="TRN2", num_swdge_queues=4)

    # all big inputs host-reordered so every DMA is fully contiguous
    hsT_d = nc.dram_tensor("hsT", [128, KO, M], bf16, kind="ExternalInput")
    wq_d = nc.dram_tensor("wqr", [NCH, 128, KO, 128], bf16, kind="ExternalInput")
    wo_d = nc.dram_tensor("wor", [NWT, 128, H, WOT], bf16, kind="ExternalInput")
    vkT_d = nc.dram_tensor("vkT", [B_LOC, HKV, D, VLM], bf16, kind="ExternalInput")
    vkTs_d = nc.dram_tensor("vkTs", [B_LOC, HKV, D, VLM], bf16, kind="ExternalInput")
    vv_d = nc.dram_tensor("vvr", [B_LOC, HKV, 128, KO, D], bf16, kind="ExternalInput")
    maskT_d = nc.dram_tensor("maskT", [Q, B_LOC, Q], f32, kind="ExternalInput")
    sel_d = nc.dram_tensor("sel", [NGRP, NGRP * 128], bf16, kind="ExternalInput")
    cos_d = nc.dram_tensor("cosT", [D, KV], bf16, kind="ExternalInput")
    sins_d = nc.dram_tensor("sinTs", [D, KV], bf16, kind="ExternalInput")
    cosq_d = nc.dram_tensor("cosqT", [D, B_LOC, Q], bf16, kind="ExternalInput")
    sinq_d = nc.dram_tensor("sinqTs", [D, B_LOC, Q], bf16, kind="ExternalInput")
    out_d = nc.dram_tensor("out", [B_LOC, Q, HID], f32, kind="ExternalOutput")

    from contextlib import ExitStack
    with tile.TileContext(nc) as tc, ExitStack() as ctx:
        const = ctx.enter_context(tc.tile_pool(name="const", bufs=1))
        wqp = ctx.enter_context(tc.tile_pool(name="wq", bufs=5))
        wop = ctx.enter_context(tc.tile_pool(name="wo", bufs=3))
        ktp = ctx.enter_context(tc.tile_pool(name="kt", bufs=3))
        ktsp = ctx.enter_context(tc.tile_pool(name="kts", bufs=3))
        krotp = ctx.enter_context(tc.tile_pool(name="krot", bufs=3))
        ktmpp = ctx.enter_context(tc.tile_pool(name="ktmp", bufs=2))
        vvp = ctx.enter_context(tc.tile_pool(name="vv", bufs=2))
        expp = ctx.enter_context(tc.tile_pool(name="expp", bufs=2))
        tmp = ctx.enter_context(tc.tile_pool(name="tmp", bufs=6))
        outp = ctx.enter_context(tc.tile_pool(name="outp", bufs=2))
        ps = ctx.enter_context(tc.tile_pool(name="ps", bufs=1, space="PSUM"))

        # one-time constants
        ones_sb = const.tile([128, 1], bf16, tag="ones")
        nc.vector.memset(ones_sb, 1.0)
        onesrow_sb = const.tile([1, 128], bf16, tag="onesrow")
        nc.vector.memset(onesrow_sb, 1.0)
        id_sb = const.tile([128, 128], bf16, tag="ident")
        make_identity(nc, id_sb)
        onesq_sb = const.tile([97, 128], bf16, tag="onesq")
        nc.vector.memset(onesq_sb, 1.0)

        # PE warm-up: dummy matmuls on the on-chip identity while startup
        # DMAs are in flight (fills the initial PE hole and carries HAM ramp)
        warm_ps = ps.tile([128, 128], f32, tag="qkv", bufs=2)
        for _ in range(40):
            nc.tensor.matmul(warm_ps, id_sb, id_sb, start=True, stop=True)

        def emit_body():
            # ------- startup loads (first weight tiles prefetched) -------
            wq_tiles = {}
            for nch in range(2):
                wq = wqp.tile([128, KO, 128], bf16, tag="wq")
                nc.sync.dma_start(out=wq, in_=wq_d[nch])
                wq_tiles[nch] = wq
            hs_sb = const.tile([128, KO, M], bf16, tag="hs")
            nc.sync.dma_start(out=hs_sb, in_=hsT_d[:])

            cos_sb = const.tile([128, KV], bf16, tag="cos")
            nc.sync.dma_start(out=cos_sb, in_=cos_d[:])
            sins_sb = const.tile([128, KV], bf16, tag="sins")
            nc.sync.dma_start(out=sins_sb, in_=sins_d[:])
            cosq_sb = const.tile([128, B_LOC, Q], bf16, tag="cosq")
            nc.sync.dma_start(out=cosq_sb, in_=cosq_d[:])
            sinq_sb = const.tile([128, B_LOC, Q], bf16, tag="sinq")
            nc.sync.dma_start(out=sinq_sb, in_=sinq_d[:])

            maskT_sb = const.tile([128, B_LOC, Q], f32, tag="maskT")
            nc.sync.dma_start(out=maskT_sb, in_=maskT_d[:])
            qT_sb = const.tile([128, B_LOC, H, Q], bf16, tag="qT")
            knT_sb = const.tile([128, B_LOC, HKV, Q], bf16, tag="knT")
            vn_sb = const.tile([128, B_LOC, HKV, D], bf16, tag="vn")
            attnT_sb = const.tile([128, B_LOC, H, Q], bf16, tag="attnT")
            sums4x = []
            for st in range(4):
                s4 = const.tile([128, M], f32, tag=f"sums4_{st}")
                nc.vector.memset(s4, 1.0)
                sums4x.append(s4)
            sel_sb = const.tile([NGRP, NGRP, 128], bf16, tag="sel")
            nc.sync.dma_start(out=sel_sb, in_=sel_d.ap().rearrange("p (g m) -> p g m", g=NGRP))

            def rope_from_psum(seg, cos_ap, sins_ap, out_ap):
                w = seg.shape[-1]
                tcos = tmp.tile([128, w], bf16, tag="tcos")
                nc.vector.tensor_tensor(out=tcos, in0=seg, in1=cos_ap, op=mybir.AluOpType.mult)
                tsin = tmp.tile([128, w], bf16, tag="tsin")
                nc.vector.tensor_tensor(
                    out=tsin[0:64, :], in0=seg[64:128, :], in1=sins_ap[0:64, :],
                    op=mybir.AluOpType.mult,
                )
                nc.vector.tensor_tensor(
                    out=tsin[64:128, :], in0=seg[0:64, :], in1=sins_ap[64:128, :],
                    op=mybir.AluOpType.mult,
                )
                nc.vector.tensor_tensor(out=out_ap, in0=tcos, in1=tsin, op=mybir.AluOpType.add)

            # ---------- phase 1: qkv^T projection ----------
            for nch in range(NCH):
                hkv, slot = nch // 4, nch % 4
                if nch in wq_tiles:
                    wq = wq_tiles.pop(nch)
                else:
                    wq = wqp.tile([128, KO, 128], bf16, tag="wq")
                    nc.sync.dma_start(out=wq, in_=wq_d[nch])
                pq = ps.tile([128, M], f32, tag="qkv", bufs=2)
                for ko in range(KO):
                    nc.tensor.matmul(
                        pq, wq[:, ko, :], hs_sb[:, ko, :],
                        start=(ko == 0), stop=(ko == KO - 1),
                    )
                if slot <= 1:
                    h = hkv * G + slot
                    for b in range(B_LOC):
                        rope_from_psum(
                            pq[:, b * Q:(b + 1) * Q],
                            cosq_sb[:, b, :], sinq_sb[:, b, :],
                            qT_sb[:, b, h, :],
                        )
                elif slot == 2:
                    for b in range(B_LOC):
                        rope_from_psum(
                            pq[:, b * Q:(b + 1) * Q],
                            cos_sb[:, VLM:VLM + Q], sins_sb[:, VLM:VLM + Q],
                            knT_sb[:, b, hkv, :],
                        )
                else:
                    vt = tmp.tile([128, M], bf16, tag="vt", bufs=3)
                    nc.vector.tensor_copy(out=vt, in_=pq)
                    for b in range(B_LOC):
                        pvt = ps.tile([128, 128], bf16, tag="qkv", bufs=2)
                        nc.tensor.transpose(pvt, vt[:, b * Q:(b + 1) * Q], id_sb)
                        nc.vector.tensor_copy(out=vn_sb[:, b, hkv, :], in_=pvt)

            # ---------- phase 2: attention per (b, hkv) ----------
            for b in range(B_LOC):
                for hkv in range(HKV):
                    grp = b * HKV + hkv
                    kt = ktp.tile([128, VLM], bf16, tag="kt")
                    nc.sync.dma_start(out=kt, in_=vkT_d[b, hkv])
                    kts = ktsp.tile([128, VLM], bf16, tag="kts")
                    nc.sync.dma_start(out=kts, in_=vkTs_d[b, hkv])
                    vvt = vvp.tile([128, KO, D], bf16, tag="vv")
                    nc.sync.dma_start(out=vvt, in_=vv_d[b, hkv])

                    krot = krotp.tile([128, VLM], bf16, tag="krot")
                    nc.vector.tensor_tensor(out=krot, in0=kt, in1=cos_sb[:, 0:VLM], op=mybir.AluOpType.mult)
                    ktmp = ktmpp.tile([128, VLM], bf16, tag="ktmp")
                    nc.vector.tensor_tensor(out=ktmp, in0=kts, in1=sins_sb[:, 0:VLM], op=mybir.AluOpType.mult)
                    nc.vector.tensor_tensor(out=krot, in0=krot, in1=ktmp, op=mybir.AluOpType.add)

                    qT_ap = qT_sb[:, b, hkv * G:(hkv + 1) * G, :]
                    expT = expp.tile([128, KVCH, M], bf16, tag="expT")

                    for cc in range((KVCH + 1) // 2):
                        c0 = cc * 2
                        npair = 2 if c0 + 1 < KVCH else 1
                        pqk = ps.tile([128, 512], f32, tag="qk", bufs=3)
                        for half in range(npair):
                            c = c0 + half
                            lhsT = krot[:, c * 128:(c + 1) * 128] if c < VLM // 128 \
                                else knT_sb[:, b, hkv, :]
                            nc.tensor.matmul(
                                pqk[:, half * M:(half + 1) * M], lhsT, qT_ap,
                                start=True, stop=True,
                            )
                            if c == KVCH - 1:
                                mask_b = maskT_sb[:, b, :]
                                mask_bc = bass.AP(
                                    tensor=mask_b.tensor, offset=mask_b.offset,
                                    ap=[mask_b.ap[0], [0, G], mask_b.ap[1]],
                                )
                                seg = pqk[:, half * M:(half + 1) * M]
                                nc.vector.tensor_tensor(out=seg, in0=seg, in1=mask_bc, op=mybir.AluOpType.add)
                        nc.scalar.activation(
                            out=expT[:, c0:c0 + npair, :], in_=pqk[:, 0:npair * M],
                            func=mybir.ActivationFunctionType.Exp,
                        )

                    po = ps.tile([128, M], f32, tag="pv", bufs=2)
                    psum_s = ps.tile([128, M], f32, tag="sum", bufs=1)
                    for c in range(KVCH):
                        lhsT = vvt[:, c, :] if c < VLM // 128 else vn_sb[:, b, hkv, :]
                        nc.tensor.matmul(po, lhsT, expT[:, c, :], start=(c == 0), stop=(c == KVCH - 1))
                    # exp-sums: 4 col-tiled M=1 accumulations run concurrently in the
                    # PE array; chunk-major order so adjacent MMs hit different col groups
                    for c in range(KVCH):
                        j = c % 4
                        nc.tensor.matmul(
                            psum_s[32 * j:32 * j + 1, :], ones_sb, expT[:, c, :],
                            start=(c < 4), stop=(c >= KVCH - 4),
                            tile_position=(0, 32 * j),
                        )
                    nc.vector.tensor_copy(out=attnT_sb[:, b, hkv * G:(hkv + 1) * G, :], in_=po)
                    scomb = tmp.tile([1, M], f32, tag="scomb", bufs=2)
                    nc.vector.tensor_copy(out=scomb, in_=psum_s[0:1, :])
                    for j in (1, 2):
                        nc.vector.tensor_tensor(out=scomb, in0=psum_s[32 * j:32 * j + 1, :], in1=scomb, op=mybir.AluOpType.add)
                    row = 32 * (grp % 4)
                    nc.vector.tensor_tensor(
                        out=sums4x[grp // 4][row:row + 1, :],
                        in0=psum_s[96:97, :], in1=scomb, op=mybir.AluOpType.add)

            # ---- softmax normalization: 4 parallel DVE reciprocals (3 of 4
            # overlap phase 2 since each tile completes after its 4 groups) ----
            rec4x = []
            for st in range(4):
                r4 = tmp.tile([128, M], f32, tag=f"rec4_{st}", bufs=1)
                nc.vector.reciprocal(out=r4, in_=sums4x[st])
                r4b = tmp.tile([128, M], bf16, tag=f"rec4b_{st}", bufs=1)
                nc.vector.tensor_copy(out=r4b, in_=r4)
                rec4x.append(r4b)
            for b in range(B_LOC):
                for hkv in range(HKV):
                    grp = b * HKV + hkv
                    row = 32 * (grp % 4)
                    prec = ps.tile([128, M], f32, tag="pv", bufs=2)
                    nc.tensor.matmul(prec, onesq_sb[row:row + 1, :],
                                     rec4x[grp // 4][row:row + 1, :],
                                     start=True, stop=True, tile_position=(row, 0))
                    at = attnT_sb[:, b, hkv * G:(hkv + 1) * G, :]
                    nc.vector.tensor_tensor(out=at, in0=at, in1=prec, op=mybir.AluOpType.mult)

            # ---------- phase 3: output projection ----------
            for nt in range(NWT):
                wo_t = wop.tile([128, H, WOT], bf16, tag="wo")
                nc.sync.dma_start(out=wo_t, in_=wo_d[nt])
                for b in range(B_LOC):
                    pw = ps.tile([128, WOT], f32, tag="qk", bufs=3)
                    for h in range(H):
                        nc.tensor.matmul(
                            pw, attnT_sb[:, b, h, :], wo_t[:, h, :],
                            start=(h == 0), stop=(h == H - 1),
                        )
                    ot = outp.tile([128, WOT], f32, tag="ot")
                    nc.vector.tensor_copy(out=ot, in_=pw)
                    nc.sync.dma_start(out=out_d[b, :, nt * WOT:(nt + 1) * WOT], in_=ot)

        for _rep in range(repeat):
            emit_body()

    nc.finalize()
    return nc


_NC_CACHE = None


def _get_nc():
    global _NC_CACHE
    if _NC_CACHE is None:
        _NC_CACHE = _build_nc()
    return _NC_CACHE


def _host_prep(hidden_states, vlm_key, vlm_value, position_ids, attention_mask,
               wqkv_w, wo_w):
    hs = np.asarray(hidden_states, dtype=np.float32)
    vk = np.asarray(vlm_key, dtype=np.float32)
    vv = np.asarray(vlm_value, dtype=np.float32)
    pos = np.asarray(position_ids).astype(np.int64)
    am = np.asarray(attention_mask, dtype=np.float32)
    wqkv = np.asarray(wqkv_w, dtype=np.float32)
    wo = np.asarray(wo_w, dtype=np.float32)

    # wqkv^T reordered: (nch, ki, ko, nj) fully contiguous per chunk
    wqkvT = wqkv.T.astype(BF)                                  # (2048, 4096)
    wq_r = np.ascontiguousarray(
        wqkvT.reshape(KO, 128, NCH, 128).transpose(2, 1, 0, 3))  # (32,128,16,128)
    # wo^T reordered: (nt, d, h, n)
    woT = wo.T.astype(BF)                                      # (hd, n)
    wo_r = np.ascontiguousarray(
        woT.reshape(H, 128, NWT, WOT).transpose(2, 1, 0, 3))     # (8,128,16,256)

    inv = 1.0 / (THETA ** (np.arange(0, D, 2, dtype=np.float32) / D))
    t = np.arange(KV, dtype=np.float32)
    fr = np.outer(t, inv)
    emb = np.concatenate([fr, fr], axis=-1)
    cosT = np.ascontiguousarray(np.cos(emb).T)       # (D, KV) fp32
    sinT = np.ascontiguousarray(np.sin(emb).T)
    sinTs = sinT.copy()
    sinTs[: D // 2] *= -1.0
    scale = 1.0 / np.sqrt(np.float32(D))

    sel = np.zeros((NGRP, NGRP * 128), dtype=np.float32)
    for g in range(NGRP):
        sel[g, g * 128:(g + 1) * 128] = 1.0
    sel = sel.astype(BF)

    in_maps = []
    for core in range(N_CORES):
        bs = slice(core * B_LOC, (core + 1) * B_LOC)
        hsT_i = np.ascontiguousarray(
            hs[bs].transpose(2, 0, 1).reshape(KO, 128, M).transpose(1, 0, 2)
        ).astype(BF)                                  # (128, 16, 256)
        vkT_i = np.ascontiguousarray(vk[bs].transpose(0, 1, 3, 2)).astype(BF)
        vkTs_i = np.ascontiguousarray(
            np.concatenate([vkT_i[:, :, D // 2:, :], vkT_i[:, :, : D // 2, :]], axis=2))
        vv_i = np.ascontiguousarray(
            vv[bs].reshape(B_LOC, HKV, KO, 128, D).transpose(0, 1, 3, 2, 4)
        ).astype(BF)                                  # (2,8,128,16,128)
        maskT_i = np.ascontiguousarray(
            np.maximum(am[bs, 0, :, VLM:], -30.0).transpose(2, 0, 1)
        ).astype(np.float32)
        posq = pos[bs] + KV - Q
        cosq_i = np.ascontiguousarray((cosT[:, posq] * scale)).astype(BF)   # (128,2,128)
        sinq_i = np.ascontiguousarray((sinTs[:, posq] * scale)).astype(BF)
        in_maps.append({
            "hsT": hsT_i,
            "wqr": wq_r,
            "wor": wo_r,
            "vkT": vkT_i,
            "vkTs": vkTs_i,
            "vvr": vv_i,
            "maskT": maskT_i,
            "sel": sel,
            "cosT": cosT.astype(BF),
            "sinTs": sinTs.astype(BF),
            "cosqT": cosq_i,
            "sinqTs": sinq_i,
        })
    return in_maps


def kernel(hidden_states, vlm_key, vlm_value, position_ids, attention_mask,
           wqkv_w, wo_w, _trace=False):
    nc = _get_nc()
    in_maps = _host_prep(hidden_states, vlm_key, vlm_value, position_ids,
                         attention_mask, wqkv_w, wo_w)
    res = run_bass_kernel_spmd(nc, in_maps, core_ids=list(range(N_CORES)), trace=_trace)
    out = np.concatenate([res.results[i]["out"] for i in range(N_CORES)], axis=0)
    if _trace:
        kernel._last_results = res
    return out.astype(np.float32)


if __name__ == "__main__":
    rng = np.random.default_rng(0)
    ins = {
        "hidden_states": rng.standard_normal((B, Q, HID), dtype=np.float32),
        "vlm_key": rng.standard_normal((B, HKV, VLM, D), dtype=np.float32),
        "vlm_value": rng.standard_normal((B, HKV, VLM, D), dtype=np.float32),
        "position_ids": np.tile(np.arange(Q, dtype=np.int32), (B, 1)),
        "attention_mask": np.zeros((B, 1, Q, KV), dtype=np.float32),
        "wqkv_w": rng.standard_normal((NQKV, HID), dtype=np.float32) * 0.02,
        "wo_w": rng.standard_normal((HID, HID), dtype=np.float32) * 0.02,
    }
    out = kernel(**ins)
    print("out", out.shape, out.dtype, float(np.abs(out).max()))
